# revision 12
# baseline (speedup 1.0000x reference)
"""Trainium2 Bass kernel for nn_AthenaSA: RMSNorm -> fused QKV -> RoPE ->
causal SDPA -> out_proj + residual, returning (out, present_k, present_v).

Sharding (8 cores): batch (2-way data parallel) x heads (4-way tensor
parallel).  Core c handles batch b=c//4 and heads [4g, 4g+4), g=c%4.  Each
core computes its 4 heads end-to-end; the out_proj partial sums are
reduce-scattered over each batch group of 4 cores, the residual is added to
the owned S/4 slice, and the host reassembles full outputs.

Compute is bf16 on the TensorEngine (fp32 PSUM accumulate); softmax sums and
normalization are fp32.  Softmax skips max-subtraction (scores are O(10) for
this problem's scale, safe in fp32 exp).

Attention layout trick: scores are computed TRANSPOSED (s_k on partitions,
s_q on free axis) so no transposes are needed anywhere: QK^T uses k_rot^T
tiles as the stationary operand, softmax denominators come from an extra
all-ones matmul accumulated alongside PV, and PV consumes v in natural
layout as the stationary operand, producing attn_out^T which feeds out_proj
directly.
"""
import math
import sys

import numpy as np

sys.path.insert(0, "/opt/trn_rl_repo")

import concourse.bass as bass  # noqa: E402
import concourse.tile as tile  # noqa: E402
from concourse import bacc, mybir  # noqa: E402
from concourse.bass_utils import run_bass_kernel_spmd  # noqa: E402

B, S, E, H, K, V = 2, 2048, 2048, 16, 128, 128
EPS = 1e-5
HL = 4            # heads per core
P = 128           # partitions
ET = E // P       # 16 e-tiles
ST = S // P       # 16 s-tiles
SB = 512          # s-block (psum free width)
NSB = S // SB     # 4 s-blocks
QC = HL * K       # 512 local qkv cols for each of q/k/v
CH = 256          # RS chunk rows
NCH = S // CH     # 8 RS chunks
F32 = mybir.dt.float32
BF16 = mybir.dt.bfloat16

_CACHE = {}
LAST_EXEC_NS = None
LAST_RESULTS = None


def _ensure_ntff_hook():
    """The image's antenv lacks axon_hooks; install an equivalent shim so
    run_bass_kernel_spmd(trace=True) can capture NTFF profiles."""
    import types
    try:
        from antenv.axon_hooks import get_axon_ntff_profile_hook  # noqa: F401
        return
    except ImportError:
        pass
    try:
        import antenv
        from trn_agent_boot.trn_boot import _ntff_profile_via_ctypes
        m = types.ModuleType("antenv.axon_hooks")
        m._hook = _ntff_profile_via_ctypes("/opt/axon/libaxon_pjrt.so")
        m.set_axon_ntff_profile_hook = lambda h: setattr(m, "_hook", h)
        m.get_axon_ntff_profile_hook = lambda: m._hook
        sys.modules["antenv.axon_hooks"] = m
        antenv.axon_hooks = m
    except Exception:
        pass


def build_graph(causal_tril: bool):
    nc = bacc.Bacc("TRN2", target_bir_lowering=False, debug=False, num_devices=8)

    embT = nc.dram_tensor("embT", [E, S], F32, kind="ExternalInput").ap()
    emb_res = nc.dram_tensor("emb_res", [NCH, CH // 4, E], F32, kind="ExternalInput").ap()
    w_qkv_t = nc.dram_tensor("w_qkv_t", [E, 3 * QC], F32, kind="ExternalInput").ap()
    w_out_s = nc.dram_tensor("w_out_s", [QC, E], F32, kind="ExternalInput").ap()
    w_norm_t = nc.dram_tensor("w_norm_t", [P, ET], F32, kind="ExternalInput").ap()
    cos_q = nc.dram_tensor("cos_q", [K, S], F32, kind="ExternalInput").ap()
    sin_q = nc.dram_tensor("sin_q", [K, S], F32, kind="ExternalInput").ap()
    cos_k = nc.dram_tensor("cos_k", [K, S], F32, kind="ExternalInput").ap()
    sin_k = nc.dram_tensor("sin_k", [K, S], F32, kind="ExternalInput").ap()
    if not causal_tril:
        maskT = nc.dram_tensor("maskT", [S, S], F32, kind="ExternalInput").ap()

    out_slice = nc.dram_tensor("out_slice", [NCH, CH // 4, E], F32,
                               kind="ExternalOutput").ap()
    k_out = nc.dram_tensor("k_out", [HL, K, S], F32, kind="ExternalOutput").ap()
    v_out = nc.dram_tensor("v_out", [S, QC], F32, kind="ExternalOutput").ap()

    inv_sqrt_k = 1.0 / math.sqrt(K)
    half = K // 2

    with tile.TileContext(nc) as tc:
        with (
            tc.tile_pool(name="dram", bufs=1, space="DRAM") as dram,
            tc.tile_pool(name="persist", bufs=1) as persist,
            tc.tile_pool(name="qkvout", bufs=1) as qkvout,
            tc.tile_pool(name="scalep", bufs=1) as scalep,
        ):
            partial = dram.tile([NCH, CH, E], F32, tag="partial")
            rs_out_d = dram.tile([NCH, CH // 4, E], F32, tag="rs_out")
            scale_d = dram.tile([1, S], F32, tag="scale_d")

            ones_bf = persist.tile([P, P], BF16, tag="ones")
            nc.vector.memset(ones_bf, 1.0)
            eps_t = persist.tile([P, 1], F32, tag="eps")
            nc.vector.memset(eps_t, EPS)
            wn_sb = persist.tile([P, ET], F32, tag="wn")
            nc.sync.dma_start(out=wn_sb, in_=w_norm_t[:, :])
            scale_col = persist.tile([P, ST], F32, tag="scale_col")

            # RMS scale, broadcast across partitions: scale_bc[p, s] = rsqrt(ms[s])
            scale_bc = scalep.tile([P, S], F32, tag="scale")

            # q/k arrive transposed [head_dim, S]; RoPE is applied IN PLACE.
            q_bf = [qkvout.tile([P, S], BF16, tag=f"qbf{j}", name=f"qbf{j}")
                    for j in range(HL)]
            k_bf = [qkvout.tile([P, S], BF16, tag=f"kbf{j}", name=f"kbf{j}")
                    for j in range(HL)]
            v_bf = [qkvout.tile([P, QC], BF16, tag=f"vbf{m}", name=f"vbf{m}")
                    for m in range(ST)]

            # ====== Phases A+B: stats + QKV (scale factored out of matmuls),
            # ====== with RoPE interleaved against the v-projection.
            with tc.tile_pool(name="xpool", bufs=1) as xpool, \
                 tc.tile_pool(name="trig", bufs=1) as trig:
                cq = trig.tile([K, S], BF16, tag="cq")
                sq_ = trig.tile([K, S], BF16, tag="sq_")
                ck = trig.tile([K, S], BF16, tag="ck")
                sk = trig.tile([K, S], BF16, tag="sk")

                x = []
                with tc.tile_pool(name="xstage", bufs=2) as xstage, \
                     tc.tile_pool(name="sqpool", bufs=3) as sqpool, \
                     tc.tile_pool(name="mspool", bufs=1, space="PSUM") as mspool:
                    for dst, srcdram in ((cq, cos_q), (sq_, sin_q),
                                         (ck, cos_k), (sk, sin_k)):
                        ts_ = xstage.tile([K, S], F32, tag="xs", name="ts_")
                        nc.sync.dma_start(out=ts_, in_=srcdram[:, :])
                        nc.gpsimd.tensor_copy(out=dst, in_=ts_)

                    ms_ps = [mspool.tile([P, SB], F32, tag=f"ms{q}", name=f"ms{q}")
                             for q in range(NSB)]
                    for e in range(ET):
                        xs = xstage.tile([P, S], F32, tag="xs")
                        nc.sync.dma_start(out=xs, in_=embT[e * P:(e + 1) * P, :])
                        xt = xpool.tile([P, S], BF16, tag=f"x{e}", name=f"x{e}")
                        nc.vector.tensor_copy(out=xt, in_=xs)
                        x.append(xt)
                        sq = sqpool.tile([P, S], BF16, tag="sq")
                        nc.scalar.activation(out=sq, in_=xs,
                                             func=mybir.ActivationFunctionType.Square)
                        for q in range(NSB):
                            nc.tensor.matmul(ms_ps[q][:, :], ones_bf,
                                             sq[:, q * SB:(q + 1) * SB],
                                             start=(e == 0), stop=(e == ET - 1))

                    for q in range(NSB):
                        rms = sqpool.tile([P, SB], F32, tag="rms", name="rms")
                        nc.scalar.activation(out=rms, in_=ms_ps[q][:, :],
                                             func=mybir.ActivationFunctionType.Sqrt,
                                             bias=eps_t, scale=1.0 / E)
                        nc.vector.reciprocal_approx_fast(
                            out=scale_bc[:, q * SB:(q + 1) * SB], in_=rms)
                    # scale in column layout (scale_col[p, m] = scale[m*128+p])
                    # via a small DRAM round trip
                    nc.sync.dma_start(out=scale_d[:, :], in_=scale_bc[0:1, :])
                    nc.sync.dma_start(
                        out=scale_col,
                        in_=scale_d[0].rearrange("(t p) -> p t", p=P))

                # ---- QKV projections; w_qkv streamed in 512-col slices with
                # ---- w_norm folded into the bf16 cast
                with tc.tile_pool(name="qkps", bufs=3, space="PSUM") as qkps, \
                     tc.tile_pool(name="qkcp", bufs=3) as qkcp, \
                     tc.tile_pool(name="ropetmp", bufs=2) as ropetmp:
                    def rope_inplace(src, c_, s_):
                        t1 = ropetmp.tile([K, S], BF16, tag="t1", name="t1")
                        nc.vector.tensor_copy(out=t1[0:half, :], in_=src[half:K, :])
                        nc.vector.tensor_copy(out=t1[half:K, :], in_=src[0:half, :])
                        nc.vector.tensor_mul(t1, t1, s_)
                        nc.vector.tensor_mul(src, src, c_)
                        nc.vector.tensor_add(src, src, t1)

                    for sl in range(3):      # 0: q cols, 1: k cols, 2: v cols
                        with tc.tile_pool(name=f"wp{sl}", bufs=1) as wpool:
                            w_bf = []
                            for e in range(ET):
                                ws = qkcp.tile([P, QC], F32, tag="wstage",
                                               name="ws")
                                nc.sync.dma_start(
                                    out=ws,
                                    in_=w_qkv_t[e * P:(e + 1) * P,
                                                sl * QC:(sl + 1) * QC])
                                wt = wpool.tile([P, QC], BF16, tag=f"w{e}",
                                                name=f"w{e}")
                                nc.gpsimd.tensor_scalar_mul(wt, ws,
                                                            wn_sb[:, e:e + 1])
                                w_bf.append(wt)
                            if sl < 2:
                                dsts = q_bf if sl == 0 else k_bf
                                for j in range(HL):
                                    for q in range(NSB):
                                        sslc = slice(q * SB, (q + 1) * SB)
                                        ps = qkps.tile([P, SB], F32, tag="qk")
                                        for e in range(ET):
                                            nc.tensor.matmul(
                                                ps[:, :],
                                                w_bf[e][:, j * P:(j + 1) * P],
                                                x[e][:, sslc],
                                                start=(e == 0), stop=(e == ET - 1))
                                        nc.vector.tensor_mul(
                                            dsts[j][:, sslc], ps[:, :],
                                            scale_bc[:, sslc])
                                        if sl == 1:
                                            kf = qkcp.tile([P, SB], F32,
                                                           tag="kf", name="kf")
                                            nc.vector.tensor_mul(
                                                kf, ps[:, :], scale_bc[:, sslc])
                                            nc.sync.dma_start(
                                                out=k_out[j][:, sslc], in_=kf)
                                    if sl == 0:
                                        rope_inplace(q_bf[j], cq, sq_)
                                    else:
                                        rope_inplace(k_bf[j], ck, sk)
                            else:
                                for m in range(ST):
                                    ps = qkps.tile([P, QC], F32, tag="qk")
                                    for e in range(ET):
                                        nc.tensor.matmul(
                                            ps[:, :],
                                            x[e][:, m * P:(m + 1) * P],
                                            w_bf[e][:, :],
                                            start=(e == 0), stop=(e == ET - 1))
                                    vf = qkcp.tile([P, QC], F32, tag="vf")
                                    nc.vector.tensor_scalar_mul(
                                        vf, ps[:, :], scale_col[:, m:m + 1])
                                    nc.sync.dma_start(
                                        out=v_out[m * P:(m + 1) * P, :], in_=vf)
                                    nc.vector.tensor_copy(out=v_bf[m], in_=vf)

            # ====== Phase D: attention + out_proj + chunked RS + residual ===
            with tc.tile_pool(name="wout", bufs=1) as woutp, \
                 tc.tile_pool(name="maskp", bufs=1) as maskp, \
                 tc.tile_pool(name="attn", bufs=1) as attnp, \
                 tc.tile_pool(name="expp", bufs=4) as expp, \
                 tc.tile_pool(name="smallp", bufs=4) as smallp, \
                 tc.tile_pool(name="qk2ps", bufs=2, space="PSUM") as qk2ps, \
                 tc.tile_pool(name="pvps", bufs=2, space="PSUM") as pvps, \
                 tc.tile_pool(name="sumps", bufs=2, space="PSUM") as sumps, \
                 tc.tile_pool(name="opps", bufs=2, space="PSUM") as opps, \
                 tc.tile_pool(name="finp", bufs=2) as finp:

                w_out_bf = []
                for j in range(HL):
                    wos = finp.tile([P, E], F32, tag="wostage", name="wos")
                    nc.sync.dma_start(out=wos, in_=w_out_s[j * P:(j + 1) * P, :])
                    wt = woutp.tile([P, E], BF16, tag=f"wo{j}", name=f"wo{j}")
                    nc.gpsimd.tensor_copy(out=wt, in_=wos)
                    w_out_bf.append(wt)

                if not causal_tril:
                    mask_bf = []
                    for t in range(ST):
                        mts = finp.tile([P, S], F32, tag="maskstage", name="mts")
                        nc.sync.dma_start(out=mts, in_=maskT[t * P:(t + 1) * P, :])
                        mt = maskp.tile([P, S], BF16, tag=f"mask{t}",
                                        name=f"mt{t}")
                        nc.gpsimd.tensor_copy(out=mt, in_=mts)
                        mask_bf.append(mt)

                attn_bf = [[attnp.tile([P, SB], BF16, tag=f"attn{j}_{q}",
                                       name=f"attn{j}_{q}")
                            for q in range(NSB)] for j in range(HL)]

                for Q in range(NSB):
                    nt = 4 * Q + 4 if causal_tril else ST
                    for j in range(HL):
                        pv = pvps.tile([P, SB], F32, tag="pv")
                        sm = sumps.tile([P, SB], F32, tag="sm")
                        for t in range(nt):
                            qk = qk2ps.tile([P, SB], F32, tag="qk2")
                            nc.tensor.matmul(
                                qk[:, :],
                                k_bf[j][:, t * P:(t + 1) * P],
                                q_bf[j][:, Q * SB:(Q + 1) * SB],
                                start=True, stop=True)
                            ex = expp.tile([P, SB], BF16, tag="ex")
                            nc.scalar.activation(
                                out=ex, in_=qk[:, :],
                                func=mybir.ActivationFunctionType.Exp,
                                scale=inv_sqrt_k)
                            if causal_tril:
                                if t >= 4 * Q:
                                    nc.gpsimd.affine_select(
                                        out=ex, in_=ex, pattern=[[1, SB]],
                                        compare_op=mybir.AluOpType.is_ge,
                                        fill=0.0, base=-P * (t - 4 * Q),
                                        channel_multiplier=-1)
                            else:
                                nc.vector.tensor_mul(
                                    ex, ex, mask_bf[t][:, Q * SB:(Q + 1) * SB])
                            nc.tensor.matmul(pv[:, :],
                                             v_bf[t][:, j * K:(j + 1) * K], ex,
                                             start=(t == 0), stop=(t == nt - 1))
                            nc.tensor.matmul(sm[:, :], ones_bf, ex,
                                             start=(t == 0), stop=(t == nt - 1))
                        rc = smallp.tile([P, SB], F32, tag="rc")
                        nc.vector.reciprocal_approx_fast(out=rc, in_=sm[:, :])
                        nc.vector.tensor_mul(attn_bf[j][Q], pv[:, :], rc)

                    # out_proj rows [512Q, 512Q+512); RS fires every 256 rows
                    for m in range(NSB):
                        for eb in range(NSB):
                            op = opps.tile([P, SB], F32, tag="op")
                            for j in range(HL):
                                nc.tensor.matmul(
                                    op[:, :],
                                    attn_bf[j][Q][:, m * P:(m + 1) * P],
                                    w_out_bf[j][:, eb * SB:(eb + 1) * SB],
                                    start=(j == 0), stop=(j == HL - 1))
                            ob = smallp.tile([P, SB], F32, tag="ob")
                            nc.vector.tensor_copy(out=ob, in_=op[:, :])
                            ch = (Q * SB + m * P) // CH
                            row = (Q * SB + m * P) % CH
                            nc.sync.dma_start(
                                out=partial[ch, row:row + P,
                                            eb * SB:(eb + 1) * SB],
                                in_=ob)
                        if (m * P + P) % CH == 0:
                            ch = (Q * SB + m * P) // CH
                            nc.gpsimd.collective_compute(
                                "ReduceScatter",
                                mybir.AluOpType.add,
                                ins=[partial[ch]],
                                outs=[rs_out_d[ch]],
                                replica_groups=[[0, 1, 2, 3], [4, 5, 6, 7]],
                            )
                            fin = finp.tile([CH // 4, E], F32, tag="fin")
                            nc.sync.dma_start(out=fin, in_=rs_out_d[ch])
                            res = finp.tile([CH // 4, E], F32, tag="res")
                            nc.sync.dma_start(out=res, in_=emb_res[ch])
                            nc.vector.tensor_add(fin, fin, res)
                            nc.sync.dma_start(out=out_slice[ch], in_=fin)

    nc.finalize()
    return nc


def _prep_inputs(embeddings, cos_buffer, sin_buffer, causal_buffer,
                 w_norm, w_qkv, w_out, causal_tril):
    ks = K * H
    cq = np.ascontiguousarray(np.asarray(cos_buffer)[0, 0, 0].T)
    sq = np.ascontiguousarray(np.asarray(sin_buffer)[0, 0, 0].T)
    ck = np.ascontiguousarray(np.asarray(cos_buffer)[1, 0, 0].T)
    sk = np.ascontiguousarray(np.asarray(sin_buffer)[1, 0, 0].T)
    wn_t = np.ascontiguousarray(np.asarray(w_norm).reshape(ET, P).T)
    if not causal_tril:
        maskT = np.ascontiguousarray(
            np.asarray(causal_buffer)[0, 0].T.astype(np.float32))

    in_maps = []
    for c in range(8):
        b, g = c // 4, c % 4
        emb = np.asarray(embeddings)[b]
        embT = np.ascontiguousarray(emb.T)
        rq = CH // 4
        emb_res = np.stack([emb[CH * c + rq * g: CH * c + rq * (g + 1), :]
                            for c in range(NCH)])
        wq = np.asarray(w_qkv)
        w_qkv_t = np.ascontiguousarray(np.concatenate([
            wq[:, QC * g: QC * (g + 1)],
            wq[:, ks + QC * g: ks + QC * (g + 1)],
            wq[:, 2 * ks + QC * g: 2 * ks + QC * (g + 1)],
        ], axis=1))
        w_out_sl = np.ascontiguousarray(np.asarray(w_out)[QC * g: QC * (g + 1), :])
        m = dict(embT=embT, emb_res=np.ascontiguousarray(emb_res),
                 w_qkv_t=w_qkv_t, w_out_s=w_out_sl, w_norm_t=wn_t,
                 cos_q=cq, sin_q=sq, cos_k=ck, sin_k=sk)
        if not causal_tril:
            m["maskT"] = maskT
        in_maps.append(m)
    return in_maps


def kernel(embeddings, cos_buffer, sin_buffer, causal_buffer,
           w_norm, w_qkv, w_out, trace=False):
    global LAST_EXEC_NS, LAST_RESULTS
    causal = np.asarray(causal_buffer)[0, 0]
    causal_tril = bool(np.array_equal(causal, np.tril(np.ones((S, S), bool))))

    if causal_tril not in _CACHE:
        _CACHE[causal_tril] = build_graph(causal_tril)
    nc = _CACHE[causal_tril]

    in_maps = _prep_inputs(embeddings, cos_buffer, sin_buffer, causal_buffer,
                           w_norm, w_qkv, w_out, causal_tril)
    if trace:
        _ensure_ntff_hook()
    res = run_bass_kernel_spmd(nc, in_maps, core_ids=list(range(8)), trace=trace)
    LAST_EXEC_NS = res.exec_time_ns
    LAST_RESULTS = res

    out = np.empty((B, S, E), np.float32)
    present_k = np.empty((B, H, S, K), np.float32)
    present_v = np.empty((B, H, S, V), np.float32)
    for c in range(8):
        b, g = c // 4, c % 4
        r = res.results[c]
        osl = r["out_slice"]
        rq = CH // 4
        for c in range(NCH):
            out[b, CH * c + rq * g: CH * c + rq * (g + 1), :] = osl[c]
        ko = r["k_out"]          # [HL, K, S]
        vo = r["v_out"]          # [S, QC]
        for j in range(HL):
            present_k[b, HL * g + j] = ko[j].T
            present_v[b, HL * g + j] = vo[:, K * j: K * (j + 1)]
    return out, present_k, present_v


# revision 13
# speedup vs baseline: 1.5337x; 1.5337x over previous
"""Trainium2 Bass kernel for nn_AthenaSA: RMSNorm -> fused QKV -> RoPE ->
causal SDPA -> out_proj + residual, returning (out, present_k, present_v).

Sharding (8 cores): batch (2-way data parallel) x heads (4-way tensor
parallel).  Core c handles batch b=c//4 and heads [4g, 4g+4), g=c%4.  Each
core computes its 4 heads end-to-end; the out_proj partial sums are
reduce-scattered over each batch group of 4 cores, the residual is added to
the owned S/4 slice, and the host reassembles full outputs.

Compute is bf16 on the TensorEngine (fp32 PSUM accumulate); softmax sums and
normalization are fp32.  Softmax skips max-subtraction (scores are O(10) for
this problem's scale, safe in fp32 exp).

Attention layout trick: scores are computed TRANSPOSED (s_k on partitions,
s_q on free axis) so no transposes are needed anywhere: QK^T uses k_rot^T
tiles as the stationary operand, softmax denominators come from an extra
all-ones matmul accumulated alongside PV, and PV consumes v in natural
layout as the stationary operand, producing attn_out^T which feeds out_proj
directly.
"""
import math
import sys

import numpy as np

sys.path.insert(0, "/opt/trn_rl_repo")

import concourse.bass as bass  # noqa: E402
import concourse.tile as tile  # noqa: E402
from concourse import bacc, mybir  # noqa: E402
from concourse.bass_utils import run_bass_kernel_spmd  # noqa: E402

B, S, E, H, K, V = 2, 2048, 2048, 16, 128, 128
EPS = 1e-5
HL = 4            # heads per core
P = 128           # partitions
ET = E // P       # 16 e-tiles
ST = S // P       # 16 s-tiles
SB = 512          # s-block (psum free width)
NSB = S // SB     # 4 s-blocks
QC = HL * K       # 512 local qkv cols for each of q/k/v
CH = 256          # RS chunk rows
NCH = S // CH     # 8 RS chunks
F32 = mybir.dt.float32
BF16 = mybir.dt.bfloat16

_CACHE = {}
LAST_EXEC_NS = None
LAST_RESULTS = None


def _ensure_ntff_hook():
    """The image's antenv lacks axon_hooks; install an equivalent shim so
    run_bass_kernel_spmd(trace=True) can capture NTFF profiles."""
    import types
    try:
        from antenv.axon_hooks import get_axon_ntff_profile_hook  # noqa: F401
        return
    except ImportError:
        pass
    try:
        import antenv
        from trn_agent_boot.trn_boot import _ntff_profile_via_ctypes
        m = types.ModuleType("antenv.axon_hooks")
        m._hook = _ntff_profile_via_ctypes("/opt/axon/libaxon_pjrt.so")
        m.set_axon_ntff_profile_hook = lambda h: setattr(m, "_hook", h)
        m.get_axon_ntff_profile_hook = lambda: m._hook
        sys.modules["antenv.axon_hooks"] = m
        antenv.axon_hooks = m
    except Exception:
        pass


def build_graph(causal_tril: bool):
    nc = bacc.Bacc("TRN2", target_bir_lowering=False, debug=False, num_devices=8)

    embT = nc.dram_tensor("embT", [E, S], F32, kind="ExternalInput").ap()
    emb_res = nc.dram_tensor("emb_res", [NCH, CH // 4, E], F32, kind="ExternalInput").ap()
    w_qkv_t = nc.dram_tensor("w_qkv_t", [E, 3 * QC], F32, kind="ExternalInput").ap()
    w_out_s = nc.dram_tensor("w_out_s", [QC, E], F32, kind="ExternalInput").ap()
    w_norm_t = nc.dram_tensor("w_norm_t", [P, ET], F32, kind="ExternalInput").ap()
    cos_q = nc.dram_tensor("cos_q", [K, S], F32, kind="ExternalInput").ap()
    sin_q = nc.dram_tensor("sin_q", [K, S], F32, kind="ExternalInput").ap()
    cos_k = nc.dram_tensor("cos_k", [K, S], F32, kind="ExternalInput").ap()
    sin_k = nc.dram_tensor("sin_k", [K, S], F32, kind="ExternalInput").ap()
    if not causal_tril:
        maskT = nc.dram_tensor("maskT", [S, S], F32, kind="ExternalInput").ap()

    out_slice = nc.dram_tensor("out_slice", [NCH, CH // 4, E], F32,
                               kind="ExternalOutput").ap()
    k_out = nc.dram_tensor("k_out", [HL, K, S], F32, kind="ExternalOutput").ap()
    v_out = nc.dram_tensor("v_out", [S, QC], F32, kind="ExternalOutput").ap()

    inv_sqrt_k = 1.0 / math.sqrt(K)
    half = K // 2

    with tile.TileContext(nc) as tc:
        with (
            tc.tile_pool(name="dram", bufs=1, space="DRAM") as dram,
            tc.tile_pool(name="persist", bufs=1) as persist,
            tc.tile_pool(name="qkvout", bufs=1) as qkvout,
            tc.tile_pool(name="scalep", bufs=1) as scalep,
        ):
            partial = dram.tile([NCH, CH, E], BF16, tag="partial")
            rs_out_d = dram.tile([NCH, CH // 4, E], BF16, tag="rs_out")
            scale_d = dram.tile([1, S], F32, tag="scale_d")

            ones_bf = persist.tile([P, P], BF16, tag="ones")
            nc.vector.memset(ones_bf, 1.0)
            eps_t = persist.tile([P, 1], F32, tag="eps")
            nc.vector.memset(eps_t, EPS)
            wn_sb = persist.tile([P, ET], F32, tag="wn")
            nc.sync.dma_start(out=wn_sb, in_=w_norm_t[:, :])
            scale_col = persist.tile([P, ST], F32, tag="scale_col")

            # RMS scale, broadcast across partitions: scale_bc[p, s] = rsqrt(ms[s])
            scale_bc = scalep.tile([P, S], F32, tag="scale")

            # q/k arrive transposed [head_dim, S]; RoPE is applied IN PLACE.
            q_bf = [qkvout.tile([P, S], BF16, tag=f"qbf{j}", name=f"qbf{j}")
                    for j in range(HL)]
            k_bf = [qkvout.tile([P, S], BF16, tag=f"kbf{j}", name=f"kbf{j}")
                    for j in range(HL)]
            v_bf = [qkvout.tile([P, QC], BF16, tag=f"vbf{m}", name=f"vbf{m}")
                    for m in range(ST)]

            # ====== Phases A+B: stats + QKV (scale factored out of matmuls),
            # ====== with RoPE interleaved against the v-projection.
            with tc.tile_pool(name="xpool", bufs=1) as xpool, \
                 tc.tile_pool(name="trig", bufs=1) as trig:
                cq = trig.tile([K, S], BF16, tag="cq")
                sq_ = trig.tile([K, S], BF16, tag="sq_")
                ck = trig.tile([K, S], BF16, tag="ck")
                sk = trig.tile([K, S], BF16, tag="sk")

                x = []
                with tc.tile_pool(name="xstage", bufs=2) as xstage, \
                     tc.tile_pool(name="sqpool", bufs=3) as sqpool, \
                     tc.tile_pool(name="mspool", bufs=1, space="PSUM") as mspool:
                    for dst, srcdram in ((cq, cos_q), (sq_, sin_q),
                                         (ck, cos_k), (sk, sin_k)):
                        ts_ = xstage.tile([K, S], F32, tag="xs", name="ts_")
                        nc.sync.dma_start(out=ts_, in_=srcdram[:, :])
                        nc.vector.tensor_copy(out=dst, in_=ts_)

                    ms_ps = [mspool.tile([P, SB], F32, tag=f"ms{q}", name=f"ms{q}")
                             for q in range(NSB)]
                    for e in range(ET):
                        xs = xstage.tile([P, S], F32, tag="xs")
                        nc.sync.dma_start(out=xs, in_=embT[e * P:(e + 1) * P, :])
                        xt = xpool.tile([P, S], BF16, tag=f"x{e}", name=f"x{e}")
                        nc.vector.tensor_copy(out=xt, in_=xs)
                        x.append(xt)
                        sq = sqpool.tile([P, S], BF16, tag="sq")
                        nc.scalar.activation(out=sq, in_=xs,
                                             func=mybir.ActivationFunctionType.Square)
                        for q in range(NSB):
                            nc.tensor.matmul(ms_ps[q][:, :], ones_bf,
                                             sq[:, q * SB:(q + 1) * SB],
                                             start=(e == 0), stop=(e == ET - 1))

                    for q in range(NSB):
                        rms = sqpool.tile([P, SB], F32, tag="rms", name="rms")
                        nc.scalar.activation(out=rms, in_=ms_ps[q][:, :],
                                             func=mybir.ActivationFunctionType.Sqrt,
                                             bias=eps_t, scale=1.0 / E)
                        nc.vector.reciprocal_approx_fast(
                            out=scale_bc[:, q * SB:(q + 1) * SB], in_=rms)
                    # scale in column layout (scale_col[p, m] = scale[m*128+p])
                    # via a small DRAM round trip
                    nc.sync.dma_start(out=scale_d[:, :], in_=scale_bc[0:1, :])
                    nc.sync.dma_start(
                        out=scale_col,
                        in_=scale_d[0].rearrange("(t p) -> p t", p=P))

                # ---- QKV projections; w_qkv streamed in 512-col slices with
                # ---- w_norm folded into the bf16 cast
                with tc.tile_pool(name="qkps", bufs=3, space="PSUM") as qkps, \
                     tc.tile_pool(name="qkcp", bufs=3) as qkcp, \
                     tc.tile_pool(name="ropetmp", bufs=2) as ropetmp:
                    def rope_inplace(src, c_, s_):
                        t1 = ropetmp.tile([K, S], BF16, tag="t1", name="t1")
                        nc.vector.tensor_copy(out=t1[0:half, :], in_=src[half:K, :])
                        nc.vector.tensor_copy(out=t1[half:K, :], in_=src[0:half, :])
                        nc.vector.tensor_mul(t1, t1, s_)
                        nc.vector.tensor_mul(src, src, c_)
                        nc.vector.tensor_add(src, src, t1)

                    for sl in range(3):      # 0: q cols, 1: k cols, 2: v cols
                        with tc.tile_pool(name=f"wp{sl}", bufs=1) as wpool:
                            w_bf = []
                            for e in range(ET):
                                ws = qkcp.tile([P, QC], F32, tag="wstage",
                                               name="ws")
                                nc.sync.dma_start(
                                    out=ws,
                                    in_=w_qkv_t[e * P:(e + 1) * P,
                                                sl * QC:(sl + 1) * QC])
                                wt = wpool.tile([P, QC], BF16, tag=f"w{e}",
                                                name=f"w{e}")
                                nc.vector.tensor_scalar_mul(wt, ws,
                                                            wn_sb[:, e:e + 1])
                                w_bf.append(wt)
                            if sl < 2:
                                dsts = q_bf if sl == 0 else k_bf
                                for j in range(HL):
                                    for q in range(NSB):
                                        sslc = slice(q * SB, (q + 1) * SB)
                                        ps = qkps.tile([P, SB], F32, tag="qk")
                                        for e in range(ET):
                                            nc.tensor.matmul(
                                                ps[:, :],
                                                w_bf[e][:, j * P:(j + 1) * P],
                                                x[e][:, sslc],
                                                start=(e == 0), stop=(e == ET - 1))
                                        nc.vector.tensor_mul(
                                            dsts[j][:, sslc], ps[:, :],
                                            scale_bc[:, sslc])
                                        if sl == 1:
                                            kf = qkcp.tile([P, SB], F32,
                                                           tag="kf", name="kf")
                                            nc.vector.tensor_mul(
                                                kf, ps[:, :], scale_bc[:, sslc])
                                            nc.sync.dma_start(
                                                out=k_out[j][:, sslc], in_=kf)
                                    if sl == 0:
                                        rope_inplace(q_bf[j], cq, sq_)
                                    else:
                                        rope_inplace(k_bf[j], ck, sk)
                            else:
                                for m in range(ST):
                                    ps = qkps.tile([P, QC], F32, tag="qk")
                                    for e in range(ET):
                                        nc.tensor.matmul(
                                            ps[:, :],
                                            x[e][:, m * P:(m + 1) * P],
                                            w_bf[e][:, :],
                                            start=(e == 0), stop=(e == ET - 1))
                                    vf = qkcp.tile([P, QC], F32, tag="vf")
                                    nc.vector.tensor_scalar_mul(
                                        vf, ps[:, :], scale_col[:, m:m + 1])
                                    nc.sync.dma_start(
                                        out=v_out[m * P:(m + 1) * P, :], in_=vf)
                                    nc.vector.tensor_copy(out=v_bf[m], in_=vf)

            # ====== Phase D: attention + out_proj + chunked RS + residual ===
            with tc.tile_pool(name="wout", bufs=1) as woutp, \
                 tc.tile_pool(name="maskp", bufs=1) as maskp, \
                 tc.tile_pool(name="attn", bufs=1) as attnp, \
                 tc.tile_pool(name="expp", bufs=4) as expp, \
                 tc.tile_pool(name="smallp", bufs=4) as smallp, \
                 tc.tile_pool(name="qk2ps", bufs=2, space="PSUM") as qk2ps, \
                 tc.tile_pool(name="pvps", bufs=2, space="PSUM") as pvps, \
                 tc.tile_pool(name="sumps", bufs=2, space="PSUM") as sumps, \
                 tc.tile_pool(name="opps", bufs=2, space="PSUM") as opps, \
                 tc.tile_pool(name="finp", bufs=2) as finp:

                w_out_bf = []
                for j in range(HL):
                    wos = finp.tile([P, E], F32, tag="wostage", name="wos")
                    nc.sync.dma_start(out=wos, in_=w_out_s[j * P:(j + 1) * P, :])
                    wt = woutp.tile([P, E], BF16, tag=f"wo{j}", name=f"wo{j}")
                    nc.vector.tensor_copy(out=wt, in_=wos)
                    w_out_bf.append(wt)

                if not causal_tril:
                    mask_bf = []
                    for t in range(ST):
                        mts = finp.tile([P, S], F32, tag="maskstage", name="mts")
                        nc.sync.dma_start(out=mts, in_=maskT[t * P:(t + 1) * P, :])
                        mt = maskp.tile([P, S], BF16, tag=f"mask{t}",
                                        name=f"mt{t}")
                        nc.vector.tensor_copy(out=mt, in_=mts)
                        mask_bf.append(mt)

                attn_bf = [[attnp.tile([P, SB], BF16, tag=f"attn{j}_{q}",
                                       name=f"attn{j}_{q}")
                            for q in range(NSB)] for j in range(HL)]

                for Q in range(NSB):
                    nt = 4 * Q + 4 if causal_tril else ST
                    for j in range(HL):
                        pv = pvps.tile([P, SB], F32, tag="pv")
                        sm = sumps.tile([P, SB], F32, tag="sm")
                        for t in range(nt):
                            qk = qk2ps.tile([P, SB], F32, tag="qk2")
                            nc.tensor.matmul(
                                qk[:, :],
                                k_bf[j][:, t * P:(t + 1) * P],
                                q_bf[j][:, Q * SB:(Q + 1) * SB],
                                start=True, stop=True)
                            ex = expp.tile([P, SB], BF16, tag="ex")
                            nc.scalar.activation(
                                out=ex, in_=qk[:, :],
                                func=mybir.ActivationFunctionType.Exp,
                                scale=inv_sqrt_k)
                            if causal_tril:
                                if t >= 4 * Q:
                                    nc.gpsimd.affine_select(
                                        out=ex, in_=ex, pattern=[[1, SB]],
                                        compare_op=mybir.AluOpType.is_ge,
                                        fill=0.0, base=-P * (t - 4 * Q),
                                        channel_multiplier=-1)
                            else:
                                nc.vector.tensor_mul(
                                    ex, ex, mask_bf[t][:, Q * SB:(Q + 1) * SB])
                            nc.tensor.matmul(pv[:, :],
                                             v_bf[t][:, j * K:(j + 1) * K], ex,
                                             start=(t == 0), stop=(t == nt - 1))
                            nc.tensor.matmul(sm[:, :], ones_bf, ex,
                                             start=(t == 0), stop=(t == nt - 1))
                        rc = smallp.tile([P, SB], F32, tag="rc")
                        nc.vector.reciprocal_approx_fast(out=rc, in_=sm[:, :])
                        nc.vector.tensor_mul(attn_bf[j][Q], pv[:, :], rc)

                    # out_proj rows [512Q, 512Q+512); RS fires every 256 rows
                    for m in range(NSB):
                        for eb in range(NSB):
                            op = opps.tile([P, SB], F32, tag="op")
                            for j in range(HL):
                                nc.tensor.matmul(
                                    op[:, :],
                                    attn_bf[j][Q][:, m * P:(m + 1) * P],
                                    w_out_bf[j][:, eb * SB:(eb + 1) * SB],
                                    start=(j == 0), stop=(j == HL - 1))
                            ob = smallp.tile([P, SB], BF16, tag="ob")
                            nc.vector.tensor_copy(out=ob, in_=op[:, :])
                            ch = (Q * SB + m * P) // CH
                            row = (Q * SB + m * P) % CH
                            nc.sync.dma_start(
                                out=partial[ch, row:row + P,
                                            eb * SB:(eb + 1) * SB],
                                in_=ob)
                        if (m * P + P) % CH == 0:
                            ch = (Q * SB + m * P) // CH
                            nc.gpsimd.collective_compute(
                                "ReduceScatter",
                                mybir.AluOpType.add,
                                ins=[partial[ch]],
                                outs=[rs_out_d[ch]],
                                replica_groups=[[0, 1, 2, 3], [4, 5, 6, 7]],
                            )
                            fin = finp.tile([CH // 4, E], BF16, tag="fin")
                            nc.sync.dma_start(out=fin, in_=rs_out_d[ch])
                            res = finp.tile([CH // 4, E], F32, tag="res")
                            nc.sync.dma_start(out=res, in_=emb_res[ch])
                            fo = finp.tile([CH // 4, E], F32, tag="fo")
                            nc.vector.tensor_add(fo, res, fin)
                            nc.sync.dma_start(out=out_slice[ch], in_=fo)

    nc.finalize()
    return nc


def _prep_inputs(embeddings, cos_buffer, sin_buffer, causal_buffer,
                 w_norm, w_qkv, w_out, causal_tril):
    ks = K * H
    cq = np.ascontiguousarray(np.asarray(cos_buffer)[0, 0, 0].T)
    sq = np.ascontiguousarray(np.asarray(sin_buffer)[0, 0, 0].T)
    ck = np.ascontiguousarray(np.asarray(cos_buffer)[1, 0, 0].T)
    sk = np.ascontiguousarray(np.asarray(sin_buffer)[1, 0, 0].T)
    wn_t = np.ascontiguousarray(np.asarray(w_norm).reshape(ET, P).T)
    if not causal_tril:
        maskT = np.ascontiguousarray(
            np.asarray(causal_buffer)[0, 0].T.astype(np.float32))

    in_maps = []
    for c in range(8):
        b, g = c // 4, c % 4
        emb = np.asarray(embeddings)[b]
        embT = np.ascontiguousarray(emb.T)
        rq = CH // 4
        emb_res = np.stack([emb[CH * c + rq * g: CH * c + rq * (g + 1), :]
                            for c in range(NCH)])
        wq = np.asarray(w_qkv)
        w_qkv_t = np.ascontiguousarray(np.concatenate([
            wq[:, QC * g: QC * (g + 1)],
            wq[:, ks + QC * g: ks + QC * (g + 1)],
            wq[:, 2 * ks + QC * g: 2 * ks + QC * (g + 1)],
        ], axis=1))
        w_out_sl = np.ascontiguousarray(np.asarray(w_out)[QC * g: QC * (g + 1), :])
        m = dict(embT=embT, emb_res=np.ascontiguousarray(emb_res),
                 w_qkv_t=w_qkv_t, w_out_s=w_out_sl, w_norm_t=wn_t,
                 cos_q=cq, sin_q=sq, cos_k=ck, sin_k=sk)
        if not causal_tril:
            m["maskT"] = maskT
        in_maps.append(m)
    return in_maps


def kernel(embeddings, cos_buffer, sin_buffer, causal_buffer,
           w_norm, w_qkv, w_out, trace=False):
    global LAST_EXEC_NS, LAST_RESULTS
    causal = np.asarray(causal_buffer)[0, 0]
    causal_tril = bool(np.array_equal(causal, np.tril(np.ones((S, S), bool))))

    if causal_tril not in _CACHE:
        _CACHE[causal_tril] = build_graph(causal_tril)
    nc = _CACHE[causal_tril]

    in_maps = _prep_inputs(embeddings, cos_buffer, sin_buffer, causal_buffer,
                           w_norm, w_qkv, w_out, causal_tril)
    if trace:
        _ensure_ntff_hook()
    res = run_bass_kernel_spmd(nc, in_maps, core_ids=list(range(8)), trace=trace)
    LAST_EXEC_NS = res.exec_time_ns
    LAST_RESULTS = res

    out = np.empty((B, S, E), np.float32)
    present_k = np.empty((B, H, S, K), np.float32)
    present_v = np.empty((B, H, S, V), np.float32)
    for c in range(8):
        b, g = c // 4, c % 4
        r = res.results[c]
        osl = r["out_slice"]
        rq = CH // 4
        for c in range(NCH):
            out[b, CH * c + rq * g: CH * c + rq * (g + 1), :] = osl[c]
        ko = r["k_out"]          # [HL, K, S]
        vo = r["v_out"]          # [S, QC]
        for j in range(HL):
            present_k[b, HL * g + j] = ko[j].T
            present_v[b, HL * g + j] = vo[:, K * j: K * (j + 1)]
    return out, present_k, present_v


# revision 16
# speedup vs baseline: 1.5719x; 1.0249x over previous
"""Trainium2 Bass kernel for nn_AthenaSA: RMSNorm -> fused QKV -> RoPE ->
causal SDPA -> out_proj + residual, returning (out, present_k, present_v).

Sharding (8 cores): batch (2-way data parallel) x heads (4-way tensor
parallel).  Core c handles batch b=c//4 and heads [4g, 4g+4), g=c%4.  Each
core computes its 4 heads end-to-end; the out_proj partial sums are
reduce-scattered over each batch group of 4 cores, the residual is added to
the owned S/4 slice, and the host reassembles full outputs.

Compute is bf16 on the TensorEngine (fp32 PSUM accumulate); softmax sums and
normalization are fp32.  Softmax skips max-subtraction (scores are O(10) for
this problem's scale, safe in fp32 exp).

Attention layout trick: scores are computed TRANSPOSED (s_k on partitions,
s_q on free axis) so no transposes are needed anywhere: QK^T uses k_rot^T
tiles as the stationary operand, softmax denominators come from an extra
all-ones matmul accumulated alongside PV, and PV consumes v in natural
layout as the stationary operand, producing attn_out^T which feeds out_proj
directly.
"""
import math
import sys

import numpy as np

sys.path.insert(0, "/opt/trn_rl_repo")

import concourse.bass as bass  # noqa: E402
import concourse.tile as tile  # noqa: E402
from concourse import bacc, mybir  # noqa: E402
from concourse.bass_utils import run_bass_kernel_spmd  # noqa: E402

B, S, E, H, K, V = 2, 2048, 2048, 16, 128, 128
EPS = 1e-5
HL = 4            # heads per core
P = 128           # partitions
ET = E // P       # 16 e-tiles
ST = S // P       # 16 s-tiles
SB = 512          # s-block (psum free width)
NSB = S // SB     # 4 s-blocks
QC = HL * K       # 512 local qkv cols for each of q/k/v
CH = 256          # RS chunk rows
NCH = S // CH     # 8 RS chunks
F32 = mybir.dt.float32
BF16 = mybir.dt.bfloat16

_CACHE = {}
LAST_EXEC_NS = None
LAST_RESULTS = None


def _ensure_ntff_hook():
    """The image's antenv lacks axon_hooks; install an equivalent shim so
    run_bass_kernel_spmd(trace=True) can capture NTFF profiles."""
    import types
    try:
        from antenv.axon_hooks import get_axon_ntff_profile_hook  # noqa: F401
        return
    except ImportError:
        pass
    try:
        import antenv
        from trn_agent_boot.trn_boot import _ntff_profile_via_ctypes
        m = types.ModuleType("antenv.axon_hooks")
        m._hook = _ntff_profile_via_ctypes("/opt/axon/libaxon_pjrt.so")
        m.set_axon_ntff_profile_hook = lambda h: setattr(m, "_hook", h)
        m.get_axon_ntff_profile_hook = lambda: m._hook
        sys.modules["antenv.axon_hooks"] = m
        antenv.axon_hooks = m
    except Exception:
        pass


def build_graph(causal_tril: bool):
    nc = bacc.Bacc("TRN2", target_bir_lowering=False, debug=False, num_devices=8)

    embT = nc.dram_tensor("embT", [E, S], F32, kind="ExternalInput").ap()
    emb_res = nc.dram_tensor("emb_res", [NCH, CH // 4, E], F32, kind="ExternalInput").ap()
    w_qkv_t = nc.dram_tensor("w_qkv_t", [E, 3 * QC], F32, kind="ExternalInput").ap()
    w_out_s = nc.dram_tensor("w_out_s", [QC, E], F32, kind="ExternalInput").ap()
    w_norm_t = nc.dram_tensor("w_norm_t", [P, ET], F32, kind="ExternalInput").ap()
    cos_q = nc.dram_tensor("cos_q", [K, S], F32, kind="ExternalInput").ap()
    sin_q = nc.dram_tensor("sin_q", [K, S], F32, kind="ExternalInput").ap()
    cos_k = nc.dram_tensor("cos_k", [K, S], F32, kind="ExternalInput").ap()
    sin_k = nc.dram_tensor("sin_k", [K, S], F32, kind="ExternalInput").ap()
    if not causal_tril:
        maskT = nc.dram_tensor("maskT", [S, S], F32, kind="ExternalInput").ap()

    out_slice = nc.dram_tensor("out_slice", [NCH, CH // 4, E], F32,
                               kind="ExternalOutput").ap()
    k_out = nc.dram_tensor("k_out", [HL, K, S], F32, kind="ExternalOutput").ap()
    v_out = nc.dram_tensor("v_out", [S, QC], F32, kind="ExternalOutput").ap()

    inv_sqrt_k = 1.0 / math.sqrt(K)
    half = K // 2

    with tile.TileContext(nc) as tc:
        with (
            tc.tile_pool(name="dram", bufs=1, space="DRAM") as dram,
            tc.tile_pool(name="persist", bufs=1) as persist,
            tc.tile_pool(name="qkvout", bufs=1) as qkvout,
            tc.tile_pool(name="scalep", bufs=1) as scalep,
        ):
            partial = [dram.tile([CH, E], BF16, tag=f"partial{c}",
                                 name=f"partial{c}") for c in range(NCH)]
            rs_out_d = [dram.tile([CH // 4, E], BF16, tag=f"rs_out{c}",
                                  name=f"rs_out{c}") for c in range(NCH)]
            scale_d = dram.tile([1, S], F32, tag="scale_d")

            ones_bf = persist.tile([P, P], BF16, tag="ones")
            nc.vector.memset(ones_bf, 1.0)
            ones1 = persist.tile([P, 1], BF16, tag="ones1")
            nc.vector.memset(ones1, 1.0)
            eps_t = persist.tile([P, 1], F32, tag="eps")
            nc.vector.memset(eps_t, EPS)
            wn_sb = persist.tile([P, ET], F32, tag="wn")
            nc.sync.dma_start(out=wn_sb, in_=w_norm_t[:, :])
            scale_col = persist.tile([P, ST], F32, tag="scale_col")

            # RMS scale, broadcast across partitions: scale_bc[p, s] = rsqrt(ms[s])
            scale_bc = scalep.tile([P, S], F32, tag="scale")

            # q/k arrive transposed [head_dim, S]; RoPE is applied IN PLACE.
            q_bf = [qkvout.tile([P, S], BF16, tag=f"qbf{j}", name=f"qbf{j}")
                    for j in range(HL)]
            k_bf = [qkvout.tile([P, S], BF16, tag=f"kbf{j}", name=f"kbf{j}")
                    for j in range(HL)]
            v_bf = [qkvout.tile([P, QC], BF16, tag=f"vbf{m}", name=f"vbf{m}")
                    for m in range(ST)]

            # ====== Phases A+B: stats + QKV (scale factored out of matmuls),
            # ====== with RoPE interleaved against the v-projection.
            with tc.tile_pool(name="xpool", bufs=1) as xpool, \
                 tc.tile_pool(name="trig", bufs=1) as trig:
                cq = trig.tile([K, S], BF16, tag="cq")
                sq_ = trig.tile([K, S], BF16, tag="sq_")
                ck = trig.tile([K, S], BF16, tag="ck")
                sk = trig.tile([K, S], BF16, tag="sk")

                x = []
                with tc.tile_pool(name="xstage", bufs=3) as xstage, \
                     tc.tile_pool(name="sqpool", bufs=3) as sqpool, \
                     tc.tile_pool(name="mspool", bufs=1, space="PSUM") as mspool:
                    for dst, srcdram in ((cq, cos_q), (sq_, sin_q),
                                         (ck, cos_k), (sk, sin_k)):
                        ts_ = xstage.tile([K, S], F32, tag="xs", name="ts_")
                        nc.sync.dma_start(out=ts_, in_=srcdram[:, :])
                        nc.vector.tensor_copy(out=dst, in_=ts_)

                    ms_ps = [mspool.tile([P, SB], F32, tag=f"ms{q}", name=f"ms{q}")
                             for q in range(NSB)]
                    for e in range(ET):
                        xs = xstage.tile([P, S], F32, tag="xs")
                        nc.sync.dma_start(out=xs, in_=embT[e * P:(e + 1) * P, :])
                        xt = xpool.tile([P, S], BF16, tag=f"x{e}", name=f"x{e}")
                        nc.vector.tensor_copy(out=xt, in_=xs)
                        x.append(xt)
                        sq = sqpool.tile([P, S], BF16, tag="sq")
                        nc.scalar.activation(out=sq, in_=xs,
                                             func=mybir.ActivationFunctionType.Square)
                        for q in range(NSB):
                            nc.tensor.matmul(ms_ps[q][:, :], ones_bf,
                                             sq[:, q * SB:(q + 1) * SB],
                                             start=(e == 0), stop=(e == ET - 1))

                    for q in range(NSB):
                        rms = sqpool.tile([P, SB], F32, tag="rms", name="rms")
                        nc.scalar.activation(out=rms, in_=ms_ps[q][:, :],
                                             func=mybir.ActivationFunctionType.Sqrt,
                                             bias=eps_t, scale=1.0 / E)
                        nc.vector.reciprocal_approx_fast(
                            out=scale_bc[:, q * SB:(q + 1) * SB], in_=rms)
                    # scale in column layout (scale_col[p, m] = scale[m*128+p])
                    # via a small DRAM round trip
                    nc.sync.dma_start(out=scale_d[:, :], in_=scale_bc[0:1, :])
                    nc.sync.dma_start(
                        out=scale_col,
                        in_=scale_d[0].rearrange("(t p) -> p t", p=P))

                # ---- QKV projections; w_qkv streamed in 512-col slices with
                # ---- w_norm folded into the bf16 cast
                with tc.tile_pool(name="qkps", bufs=3, space="PSUM") as qkps, \
                     tc.tile_pool(name="qkcp", bufs=3) as qkcp, \
                     tc.tile_pool(name="ropetmp", bufs=2) as ropetmp:
                    def rope_inplace(src, c_, s_):
                        t1 = ropetmp.tile([K, S], BF16, tag="t1", name="t1")
                        nc.vector.tensor_copy(out=t1[0:half, :], in_=src[half:K, :])
                        nc.vector.tensor_copy(out=t1[half:K, :], in_=src[0:half, :])
                        nc.vector.tensor_mul(t1, t1, s_)
                        nc.vector.tensor_mul(src, src, c_)
                        nc.vector.tensor_add(src, src, t1)

                    for sl in range(3):      # 0: q cols, 1: k cols, 2: v cols
                        with tc.tile_pool(name=f"wp{sl}", bufs=1) as wpool:
                            w_bf = []
                            for e in range(ET):
                                ws = qkcp.tile([P, QC], F32, tag="wstage",
                                               name="ws")
                                nc.sync.dma_start(
                                    out=ws,
                                    in_=w_qkv_t[e * P:(e + 1) * P,
                                                sl * QC:(sl + 1) * QC])
                                wt = wpool.tile([P, QC], BF16, tag=f"w{e}",
                                                name=f"w{e}")
                                nc.vector.tensor_scalar_mul(wt, ws,
                                                            wn_sb[:, e:e + 1])
                                w_bf.append(wt)
                            if sl < 2:
                                dsts = q_bf if sl == 0 else k_bf
                                for j in range(HL):
                                    for q in range(NSB):
                                        sslc = slice(q * SB, (q + 1) * SB)
                                        ps = qkps.tile([P, SB], F32, tag="qk")
                                        for e in range(ET):
                                            nc.tensor.matmul(
                                                ps[:, :],
                                                w_bf[e][:, j * P:(j + 1) * P],
                                                x[e][:, sslc],
                                                start=(e == 0), stop=(e == ET - 1))
                                        nc.vector.tensor_mul(
                                            dsts[j][:, sslc], ps[:, :],
                                            scale_bc[:, sslc])
                                        if sl == 1:
                                            kf = qkcp.tile([P, SB], F32,
                                                           tag="kf", name="kf")
                                            nc.vector.tensor_mul(
                                                kf, ps[:, :], scale_bc[:, sslc])
                                            nc.sync.dma_start(
                                                out=k_out[j][:, sslc], in_=kf)
                                    if sl == 0:
                                        rope_inplace(q_bf[j], cq, sq_)
                                    else:
                                        rope_inplace(k_bf[j], ck, sk)
                            else:
                                for m in range(ST):
                                    ps = qkps.tile([P, QC], F32, tag="qk")
                                    for e in range(ET):
                                        nc.tensor.matmul(
                                            ps[:, :],
                                            x[e][:, m * P:(m + 1) * P],
                                            w_bf[e][:, :],
                                            start=(e == 0), stop=(e == ET - 1))
                                    vf = qkcp.tile([P, QC], F32, tag="vf")
                                    nc.vector.tensor_scalar_mul(
                                        vf, ps[:, :], scale_col[:, m:m + 1])
                                    nc.sync.dma_start(
                                        out=v_out[m * P:(m + 1) * P, :], in_=vf)
                                    nc.vector.tensor_copy(out=v_bf[m], in_=vf)

            # ====== Phase D: attention + out_proj + chunked RS + residual ===
            with tc.tile_pool(name="wout", bufs=1) as woutp, \
                 tc.tile_pool(name="maskp", bufs=1) as maskp, \
                 tc.tile_pool(name="attn", bufs=1) as attnp, \
                 tc.tile_pool(name="expp", bufs=4) as expp, \
                 tc.tile_pool(name="smallp", bufs=4) as smallp, \
                 tc.tile_pool(name="qk2ps", bufs=2, space="PSUM") as qk2ps, \
                 tc.tile_pool(name="pvps", bufs=2, space="PSUM") as pvps, \
                 tc.tile_pool(name="sumps", bufs=2, space="PSUM") as sumps, \
                 tc.tile_pool(name="opps", bufs=2, space="PSUM") as opps, \
                 tc.tile_pool(name="finp", bufs=2) as finp:

                w_out_bf = []
                for j in range(HL):
                    wos = finp.tile([P, E], F32, tag="wostage", name="wos")
                    nc.sync.dma_start(out=wos, in_=w_out_s[j * P:(j + 1) * P, :])
                    wt = woutp.tile([P, E], BF16, tag=f"wo{j}", name=f"wo{j}")
                    nc.vector.tensor_copy(out=wt, in_=wos)
                    w_out_bf.append(wt)

                if not causal_tril:
                    mask_bf = []
                    for t in range(ST):
                        mts = finp.tile([P, S], F32, tag="maskstage", name="mts")
                        nc.sync.dma_start(out=mts, in_=maskT[t * P:(t + 1) * P, :])
                        mt = maskp.tile([P, S], BF16, tag=f"mask{t}",
                                        name=f"mt{t}")
                        nc.vector.tensor_copy(out=mt, in_=mts)
                        mask_bf.append(mt)

                attn_bf = [[attnp.tile([P, SB], BF16, tag=f"attn{j}_{q}",
                                       name=f"attn{j}_{q}")
                            for q in range(NSB)] for j in range(HL)]

                for Q in range(NSB):
                    nt = 4 * Q + 4 if causal_tril else ST
                    for j in range(HL):
                        pv = pvps.tile([P, SB], F32, tag="pv")
                        sm = sumps.tile([P, SB], F32, tag="sm")
                        for t in range(nt):
                            qk = qk2ps.tile([P, SB], F32, tag="qk2")
                            nc.tensor.matmul(
                                qk[:, :],
                                k_bf[j][:, t * P:(t + 1) * P],
                                q_bf[j][:, Q * SB:(Q + 1) * SB],
                                start=True, stop=True)
                            ex = expp.tile([P, SB], BF16, tag="ex")
                            nc.scalar.activation(
                                out=ex, in_=qk[:, :],
                                func=mybir.ActivationFunctionType.Exp,
                                scale=inv_sqrt_k)
                            if causal_tril:
                                if t >= 4 * Q:
                                    nc.gpsimd.affine_select(
                                        out=ex, in_=ex, pattern=[[1, SB]],
                                        compare_op=mybir.AluOpType.is_ge,
                                        fill=0.0, base=-P * (t - 4 * Q),
                                        channel_multiplier=-1)
                            else:
                                nc.vector.tensor_mul(
                                    ex, ex, mask_bf[t][:, Q * SB:(Q + 1) * SB])
                            nc.tensor.matmul(pv[:, :],
                                             v_bf[t][:, j * K:(j + 1) * K], ex,
                                             start=(t == 0), stop=(t == nt - 1))
                            nc.tensor.matmul(sm[:, :], ones_bf, ex,
                                             start=(t == 0), stop=(t == nt - 1))
                        rc = smallp.tile([P, SB], F32, tag="rc")
                        nc.vector.reciprocal_approx_fast(out=rc, in_=sm[:, :])
                        nc.vector.tensor_mul(attn_bf[j][Q], pv[:, :], rc)

                    # out_proj rows [512Q, 512Q+512); RS fires every 256 rows
                    for m in range(NSB):
                        for eb in range(NSB):
                            op = opps.tile([P, SB], F32, tag="op")
                            for j in range(HL):
                                nc.tensor.matmul(
                                    op[:, :],
                                    attn_bf[j][Q][:, m * P:(m + 1) * P],
                                    w_out_bf[j][:, eb * SB:(eb + 1) * SB],
                                    start=(j == 0), stop=(j == HL - 1))
                            ob = smallp.tile([P, SB], BF16, tag="ob")
                            nc.vector.tensor_copy(out=ob, in_=op[:, :])
                            ch = (Q * SB + m * P) // CH
                            row = (Q * SB + m * P) % CH
                            nc.sync.dma_start(
                                out=partial[ch][row:row + P,
                                               eb * SB:(eb + 1) * SB],
                                in_=ob)
                        if (m * P + P) % CH == 0:
                            ch = (Q * SB + m * P) // CH
                            nc.gpsimd.collective_compute(
                                "ReduceScatter",
                                mybir.AluOpType.add,
                                ins=[partial[ch][:, :]],
                                outs=[rs_out_d[ch][:, :]],
                                replica_groups=[[0, 1, 2, 3], [4, 5, 6, 7]],
                            )
                            fin = finp.tile([CH // 4, E], BF16, tag="fin")
                            nc.sync.dma_start(out=fin, in_=rs_out_d[ch][:, :])
                            res = finp.tile([CH // 4, E], F32, tag="res")
                            nc.sync.dma_start(out=res, in_=emb_res[ch])
                            fo = finp.tile([CH // 4, E], F32, tag="fo")
                            nc.vector.tensor_add(fo, res, fin)
                            nc.sync.dma_start(out=out_slice[ch], in_=fo)

    nc.finalize()
    return nc


def _prep_inputs(embeddings, cos_buffer, sin_buffer, causal_buffer,
                 w_norm, w_qkv, w_out, causal_tril):
    ks = K * H
    cq = np.ascontiguousarray(np.asarray(cos_buffer)[0, 0, 0].T)
    sq = np.ascontiguousarray(np.asarray(sin_buffer)[0, 0, 0].T)
    ck = np.ascontiguousarray(np.asarray(cos_buffer)[1, 0, 0].T)
    sk = np.ascontiguousarray(np.asarray(sin_buffer)[1, 0, 0].T)
    wn_t = np.ascontiguousarray(np.asarray(w_norm).reshape(ET, P).T)
    if not causal_tril:
        maskT = np.ascontiguousarray(
            np.asarray(causal_buffer)[0, 0].T.astype(np.float32))

    in_maps = []
    for c in range(8):
        b, g = c // 4, c % 4
        emb = np.asarray(embeddings)[b]
        embT = np.ascontiguousarray(emb.T)
        rq = CH // 4
        emb_res = np.stack([emb[CH * c + rq * g: CH * c + rq * (g + 1), :]
                            for c in range(NCH)])
        wq = np.asarray(w_qkv)
        w_qkv_t = np.ascontiguousarray(np.concatenate([
            wq[:, QC * g: QC * (g + 1)],
            wq[:, ks + QC * g: ks + QC * (g + 1)],
            wq[:, 2 * ks + QC * g: 2 * ks + QC * (g + 1)],
        ], axis=1))
        w_out_sl = np.ascontiguousarray(np.asarray(w_out)[QC * g: QC * (g + 1), :])
        m = dict(embT=embT, emb_res=np.ascontiguousarray(emb_res),
                 w_qkv_t=w_qkv_t, w_out_s=w_out_sl, w_norm_t=wn_t,
                 cos_q=cq, sin_q=sq, cos_k=ck, sin_k=sk)
        if not causal_tril:
            m["maskT"] = maskT
        in_maps.append(m)
    return in_maps


def kernel(embeddings, cos_buffer, sin_buffer, causal_buffer,
           w_norm, w_qkv, w_out, trace=False):
    global LAST_EXEC_NS, LAST_RESULTS
    causal = np.asarray(causal_buffer)[0, 0]
    causal_tril = bool(np.array_equal(causal, np.tril(np.ones((S, S), bool))))

    if causal_tril not in _CACHE:
        _CACHE[causal_tril] = build_graph(causal_tril)
    nc = _CACHE[causal_tril]

    in_maps = _prep_inputs(embeddings, cos_buffer, sin_buffer, causal_buffer,
                           w_norm, w_qkv, w_out, causal_tril)
    if trace:
        _ensure_ntff_hook()
    res = run_bass_kernel_spmd(nc, in_maps, core_ids=list(range(8)), trace=trace)
    LAST_EXEC_NS = res.exec_time_ns
    LAST_RESULTS = res

    out = np.empty((B, S, E), np.float32)
    present_k = np.empty((B, H, S, K), np.float32)
    present_v = np.empty((B, H, S, V), np.float32)
    for c in range(8):
        b, g = c // 4, c % 4
        r = res.results[c]
        osl = r["out_slice"]
        rq = CH // 4
        for c in range(NCH):
            out[b, CH * c + rq * g: CH * c + rq * (g + 1), :] = osl[c]
        ko = r["k_out"]          # [HL, K, S]
        vo = r["v_out"]          # [S, QC]
        for j in range(HL):
            present_k[b, HL * g + j] = ko[j].T
            present_v[b, HL * g + j] = vo[:, K * j: K * (j + 1)]
    return out, present_k, present_v


# revision 17
# speedup vs baseline: 1.6396x; 1.0430x over previous
"""Trainium2 Bass kernel for nn_AthenaSA: RMSNorm -> fused QKV -> RoPE ->
causal SDPA -> out_proj + residual, returning (out, present_k, present_v).

Sharding (8 cores): batch (2-way data parallel) x heads (4-way tensor
parallel).  Core c handles batch b=c//4 and heads [4g, 4g+4), g=c%4.  Each
core computes its 4 heads end-to-end; the out_proj partial sums are
reduce-scattered over each batch group of 4 cores, the residual is added to
the owned S/4 slice, and the host reassembles full outputs.

Compute is bf16 on the TensorEngine (fp32 PSUM accumulate); softmax sums and
normalization are fp32.  Softmax skips max-subtraction (scores are O(10) for
this problem's scale, safe in fp32 exp).

Attention layout trick: scores are computed TRANSPOSED (s_k on partitions,
s_q on free axis) so no transposes are needed anywhere: QK^T uses k_rot^T
tiles as the stationary operand, softmax denominators come from an extra
all-ones matmul accumulated alongside PV, and PV consumes v in natural
layout as the stationary operand, producing attn_out^T which feeds out_proj
directly.
"""
import math
import sys

import numpy as np

sys.path.insert(0, "/opt/trn_rl_repo")

import concourse.bass as bass  # noqa: E402
import concourse.tile as tile  # noqa: E402
from concourse import bacc, mybir  # noqa: E402
from concourse.bass_utils import run_bass_kernel_spmd  # noqa: E402

B, S, E, H, K, V = 2, 2048, 2048, 16, 128, 128
EPS = 1e-5
HL = 4            # heads per core
P = 128           # partitions
ET = E // P       # 16 e-tiles
ST = S // P       # 16 s-tiles
SB = 512          # s-block (psum free width)
NSB = S // SB     # 4 s-blocks
QC = HL * K       # 512 local qkv cols for each of q/k/v
CH = 256          # RS chunk rows
NCH = S // CH     # 8 RS chunks
F32 = mybir.dt.float32
BF16 = mybir.dt.bfloat16

_CACHE = {}
LAST_EXEC_NS = None
LAST_RESULTS = None


def _ensure_ntff_hook():
    """The image's antenv lacks axon_hooks; install an equivalent shim so
    run_bass_kernel_spmd(trace=True) can capture NTFF profiles."""
    import types
    try:
        from antenv.axon_hooks import get_axon_ntff_profile_hook  # noqa: F401
        return
    except ImportError:
        pass
    try:
        import antenv
        from trn_agent_boot.trn_boot import _ntff_profile_via_ctypes
        m = types.ModuleType("antenv.axon_hooks")
        m._hook = _ntff_profile_via_ctypes("/opt/axon/libaxon_pjrt.so")
        m.set_axon_ntff_profile_hook = lambda h: setattr(m, "_hook", h)
        m.get_axon_ntff_profile_hook = lambda: m._hook
        sys.modules["antenv.axon_hooks"] = m
        antenv.axon_hooks = m
    except Exception:
        pass


def build_graph(causal_tril: bool):
    nc = bacc.Bacc("TRN2", target_bir_lowering=False, debug=False, num_devices=8)

    embT = nc.dram_tensor("embT", [E, S], F32, kind="ExternalInput").ap()
    emb_res = nc.dram_tensor("emb_res", [NCH, CH // 4, E], F32, kind="ExternalInput").ap()
    w_qkv_t = nc.dram_tensor("w_qkv_t", [E, 3 * QC], F32, kind="ExternalInput").ap()
    w_out_s = nc.dram_tensor("w_out_s", [QC, E], F32, kind="ExternalInput").ap()
    w_norm_t = nc.dram_tensor("w_norm_t", [P, ET], F32, kind="ExternalInput").ap()
    cos_q = nc.dram_tensor("cos_q", [K, S], F32, kind="ExternalInput").ap()
    sin_q = nc.dram_tensor("sin_q", [K, S], F32, kind="ExternalInput").ap()
    cos_k = nc.dram_tensor("cos_k", [K, S], F32, kind="ExternalInput").ap()
    sin_k = nc.dram_tensor("sin_k", [K, S], F32, kind="ExternalInput").ap()
    if not causal_tril:
        maskT = nc.dram_tensor("maskT", [S, S], F32, kind="ExternalInput").ap()

    out_slice = nc.dram_tensor("out_slice", [NCH, CH // 4, E], F32,
                               kind="ExternalOutput").ap()
    k_out = nc.dram_tensor("k_out", [HL, K, S], F32, kind="ExternalOutput").ap()
    v_out = nc.dram_tensor("v_out", [S, QC], F32, kind="ExternalOutput").ap()

    inv_sqrt_k = 1.0 / math.sqrt(K)
    half = K // 2

    with tile.TileContext(nc) as tc:
        with (
            tc.tile_pool(name="dram", bufs=1, space="DRAM") as dram,
            tc.tile_pool(name="persist", bufs=1) as persist,
            tc.tile_pool(name="qkvout", bufs=1) as qkvout,
            tc.tile_pool(name="scalep", bufs=1) as scalep,
        ):
            partial = [dram.tile([CH, E], BF16, tag=f"partial{c}",
                                 name=f"partial{c}") for c in range(NCH)]
            rs_out_d = [dram.tile([CH // 4, E], BF16, tag=f"rs_out{c}",
                                  name=f"rs_out{c}") for c in range(NCH)]
            scale_d = dram.tile([1, S], F32, tag="scale_d")

            ones_bf = persist.tile([P, P], BF16, tag="ones")
            nc.vector.memset(ones_bf, 1.0)
            ones1 = persist.tile([P, 1], BF16, tag="ones1")
            nc.vector.memset(ones1, 1.0)
            eps_t = persist.tile([P, 1], F32, tag="eps")
            nc.vector.memset(eps_t, EPS)
            wn_sb = persist.tile([P, ET], F32, tag="wn")
            nc.sync.dma_start(out=wn_sb, in_=w_norm_t[:, :])
            scale_col = persist.tile([P, ST], F32, tag="scale_col")

            # RMS scale, broadcast across partitions: scale_bc[p, s] = rsqrt(ms[s])
            scale_bc = scalep.tile([P, S], F32, tag="scale")

            # q/k arrive transposed [head_dim, S]; RoPE is applied IN PLACE.
            q_bf = [qkvout.tile([P, S], BF16, tag=f"qbf{j}", name=f"qbf{j}")
                    for j in range(HL)]
            k_bf = [qkvout.tile([P, S], BF16, tag=f"kbf{j}", name=f"kbf{j}")
                    for j in range(HL)]
            v_bf = [qkvout.tile([P, QC], BF16, tag=f"vbf{m}", name=f"vbf{m}")
                    for m in range(ST)]

            # ====== Phases A+B: stats + QKV (scale factored out of matmuls),
            # ====== with RoPE interleaved against the v-projection.
            with tc.tile_pool(name="xpool", bufs=1) as xpool, \
                 tc.tile_pool(name="trig", bufs=1) as trig:
                cq = trig.tile([K, S], BF16, tag="cq")
                sq_ = trig.tile([K, S], BF16, tag="sq_")
                ck = trig.tile([K, S], BF16, tag="ck")
                sk = trig.tile([K, S], BF16, tag="sk")

                x = []
                with tc.tile_pool(name="xstage", bufs=3) as xstage, \
                     tc.tile_pool(name="sqpool", bufs=3) as sqpool, \
                     tc.tile_pool(name="mspool", bufs=1, space="PSUM") as mspool:
                    for dst, srcdram in ((cq, cos_q), (sq_, sin_q),
                                         (ck, cos_k), (sk, sin_k)):
                        ts_ = xstage.tile([K, S], F32, tag="xs", name="ts_")
                        nc.sync.dma_start(out=ts_, in_=srcdram[:, :])
                        nc.vector.tensor_copy(out=dst, in_=ts_)

                    ms_ps = [mspool.tile([P, SB], F32, tag=f"ms{q}", name=f"ms{q}")
                             for q in range(NSB)]
                    for e in range(ET):
                        xs = xstage.tile([P, S], F32, tag="xs")
                        nc.sync.dma_start(out=xs, in_=embT[e * P:(e + 1) * P, :])
                        xt = xpool.tile([P, S], BF16, tag=f"x{e}", name=f"x{e}")
                        nc.vector.tensor_copy(out=xt, in_=xs)
                        x.append(xt)
                        sq = sqpool.tile([P, S], BF16, tag="sq")
                        nc.scalar.activation(out=sq, in_=xs,
                                             func=mybir.ActivationFunctionType.Square)
                        for q in range(NSB):
                            nc.tensor.matmul(ms_ps[q][:, :], ones_bf,
                                             sq[:, q * SB:(q + 1) * SB],
                                             start=(e == 0), stop=(e == ET - 1))

                    for q in range(NSB):
                        rms = sqpool.tile([P, SB], F32, tag="rms", name="rms")
                        nc.scalar.activation(out=rms, in_=ms_ps[q][:, :],
                                             func=mybir.ActivationFunctionType.Sqrt,
                                             bias=eps_t, scale=1.0 / E)
                        nc.vector.reciprocal_approx_fast(
                            out=scale_bc[:, q * SB:(q + 1) * SB], in_=rms)
                    # scale in column layout (scale_col[p, m] = scale[m*128+p])
                    # via a small DRAM round trip
                    nc.sync.dma_start(out=scale_d[:, :], in_=scale_bc[0:1, :])
                    nc.sync.dma_start(
                        out=scale_col,
                        in_=scale_d[0].rearrange("(t p) -> p t", p=P))

                # ---- QKV projections; w_qkv streamed in 512-col slices with
                # ---- w_norm folded into the bf16 cast
                with tc.tile_pool(name="qkps", bufs=3, space="PSUM") as qkps, \
                     tc.tile_pool(name="qkcp", bufs=3) as qkcp, \
                     tc.tile_pool(name="ropetmp", bufs=2) as ropetmp:
                    def rope_inplace(src, c_, s_):
                        t1 = ropetmp.tile([K, S], BF16, tag="t1", name="t1")
                        nc.vector.tensor_copy(out=t1[0:half, :], in_=src[half:K, :])
                        nc.vector.tensor_copy(out=t1[half:K, :], in_=src[0:half, :])
                        nc.vector.tensor_mul(t1, t1, s_)
                        nc.vector.tensor_mul(src, src, c_)
                        nc.vector.tensor_add(src, src, t1)

                    for sl in range(3):      # 0: q cols, 1: k cols, 2: v cols
                        with tc.tile_pool(name=f"wp{sl}", bufs=1) as wpool:
                            w_bf = []
                            for e in range(ET):
                                ws = qkcp.tile([P, QC], F32, tag="wstage",
                                               name="ws")
                                nc.sync.dma_start(
                                    out=ws,
                                    in_=w_qkv_t[e * P:(e + 1) * P,
                                                sl * QC:(sl + 1) * QC])
                                wt = wpool.tile([P, QC], BF16, tag=f"w{e}",
                                                name=f"w{e}")
                                nc.vector.tensor_scalar_mul(wt, ws,
                                                            wn_sb[:, e:e + 1])
                                w_bf.append(wt)
                            if sl < 2:
                                dsts = q_bf if sl == 0 else k_bf
                                for j in range(HL):
                                    for q in range(NSB):
                                        sslc = slice(q * SB, (q + 1) * SB)
                                        ps = qkps.tile([P, SB], F32, tag="qk")
                                        for e in range(ET):
                                            nc.tensor.matmul(
                                                ps[:, :],
                                                w_bf[e][:, j * P:(j + 1) * P],
                                                x[e][:, sslc],
                                                start=(e == 0), stop=(e == ET - 1))
                                        nc.vector.tensor_mul(
                                            dsts[j][:, sslc], ps[:, :],
                                            scale_bc[:, sslc])
                                        if sl == 1:
                                            kf = qkcp.tile([P, SB], F32,
                                                           tag="kf", name="kf")
                                            nc.vector.tensor_mul(
                                                kf, ps[:, :], scale_bc[:, sslc])
                                            nc.sync.dma_start(
                                                out=k_out[j][:, sslc], in_=kf)
                                    if sl == 0:
                                        rope_inplace(q_bf[j], cq, sq_)
                                    else:
                                        rope_inplace(k_bf[j], ck, sk)
                            else:
                                for m in range(ST):
                                    ps = qkps.tile([P, QC], F32, tag="qk")
                                    for e in range(ET):
                                        nc.tensor.matmul(
                                            ps[:, :],
                                            x[e][:, m * P:(m + 1) * P],
                                            w_bf[e][:, :],
                                            start=(e == 0), stop=(e == ET - 1))
                                    vf = qkcp.tile([P, QC], F32, tag="vf")
                                    nc.vector.tensor_scalar_mul(
                                        vf, ps[:, :], scale_col[:, m:m + 1])
                                    nc.sync.dma_start(
                                        out=v_out[m * P:(m + 1) * P, :], in_=vf)
                                    nc.vector.tensor_copy(out=v_bf[m], in_=vf)

            # ====== Phase D: attention + out_proj + chunked RS + residual ===
            with tc.tile_pool(name="wout", bufs=1) as woutp, \
                 tc.tile_pool(name="maskp", bufs=1) as maskp, \
                 tc.tile_pool(name="attn", bufs=1) as attnp, \
                 tc.tile_pool(name="expp", bufs=4) as expp, \
                 tc.tile_pool(name="smallp", bufs=4) as smallp, \
                 tc.tile_pool(name="qk2ps", bufs=2, space="PSUM") as qk2ps, \
                 tc.tile_pool(name="pvps", bufs=2, space="PSUM") as pvps, \
                 tc.tile_pool(name="sumps", bufs=2, space="PSUM") as sumps, \
                 tc.tile_pool(name="opps", bufs=2, space="PSUM") as opps, \
                 tc.tile_pool(name="finp", bufs=2) as finp:

                w_out_bf = []
                for j in range(HL):
                    wos = finp.tile([P, E], F32, tag="wostage", name="wos")
                    nc.sync.dma_start(out=wos, in_=w_out_s[j * P:(j + 1) * P, :])
                    wt = woutp.tile([P, E], BF16, tag=f"wo{j}", name=f"wo{j}")
                    nc.vector.tensor_copy(out=wt, in_=wos)
                    w_out_bf.append(wt)

                if not causal_tril:
                    mask_bf = []
                    for t in range(ST):
                        mts = finp.tile([P, S], F32, tag="maskstage", name="mts")
                        nc.sync.dma_start(out=mts, in_=maskT[t * P:(t + 1) * P, :])
                        mt = maskp.tile([P, S], BF16, tag=f"mask{t}",
                                        name=f"mt{t}")
                        nc.vector.tensor_copy(out=mt, in_=mts)
                        mask_bf.append(mt)

                attn_bf = [[attnp.tile([P, SB], BF16, tag=f"attn{j}_{q}",
                                       name=f"attn{j}_{q}")
                            for q in range(NSB)] for j in range(HL)]

                for Q in range(NSB):
                    nt = 4 * Q + 4 if causal_tril else ST
                    for j in range(HL):
                        pv = pvps.tile([P, SB], F32, tag="pv")
                        sm = sumps.tile([P, SB], F32, tag="sm")
                        for t in range(nt):
                            qk = qk2ps.tile([P, SB], F32, tag="qk2")
                            nc.tensor.matmul(
                                qk[:, :],
                                k_bf[j][:, t * P:(t + 1) * P],
                                q_bf[j][:, Q * SB:(Q + 1) * SB],
                                start=True, stop=True)
                            ex = expp.tile([P, SB], BF16, tag="ex")
                            nc.scalar.activation(
                                out=ex, in_=qk[:, :],
                                func=mybir.ActivationFunctionType.Exp,
                                scale=inv_sqrt_k)
                            if causal_tril:
                                if t >= 4 * Q:
                                    nc.gpsimd.affine_select(
                                        out=ex, in_=ex, pattern=[[1, SB]],
                                        compare_op=mybir.AluOpType.is_ge,
                                        fill=0.0, base=-P * (t - 4 * Q),
                                        channel_multiplier=-1)
                            else:
                                nc.vector.tensor_mul(
                                    ex, ex, mask_bf[t][:, Q * SB:(Q + 1) * SB])
                            nc.tensor.matmul(pv[:, :],
                                             v_bf[t][:, j * K:(j + 1) * K], ex,
                                             start=(t == 0), stop=(t == nt - 1))
                            nc.tensor.matmul(sm[:, :], ones_bf, ex,
                                             start=(t == 0), stop=(t == nt - 1))
                        rc = smallp.tile([P, SB], F32, tag="rc")
                        nc.vector.reciprocal_approx_fast(out=rc, in_=sm[:, :])
                        nc.vector.tensor_mul(attn_bf[j][Q], pv[:, :], rc)

                    # out_proj rows [512Q, 512Q+512); RS fires every 256 rows
                    for m in range(NSB):
                        for eb in range(NSB):
                            op = opps.tile([P, SB], F32, tag="op")
                            for j in range(HL):
                                nc.tensor.matmul(
                                    op[:, :],
                                    attn_bf[j][Q][:, m * P:(m + 1) * P],
                                    w_out_bf[j][:, eb * SB:(eb + 1) * SB],
                                    start=(j == 0), stop=(j == HL - 1))
                            ob = smallp.tile([P, SB], BF16, tag="ob")
                            nc.vector.tensor_copy(out=ob, in_=op[:, :])
                            ch = (Q * SB + m * P) // CH
                            row = (Q * SB + m * P) % CH
                            nc.sync.dma_start(
                                out=partial[ch][row:row + P,
                                               eb * SB:(eb + 1) * SB],
                                in_=ob)
                        if (m * P + P) % CH == 0:
                            ch = (Q * SB + m * P) // CH
                            nc.gpsimd.collective_compute(
                                "ReduceScatter",
                                mybir.AluOpType.add,
                                ins=[partial[ch][:, :]],
                                outs=[rs_out_d[ch][:, :]],
                                replica_groups=[[0, 1, 2, 3], [4, 5, 6, 7]],
                            )

                # residual + writeback after all compute: keeps the
                # RS-latency waits off the in-order engine streams
                for ch in range(NCH):
                    fin = finp.tile([CH // 4, E], BF16, tag="fin")
                    nc.sync.dma_start(out=fin, in_=rs_out_d[ch][:, :])
                    res = finp.tile([CH // 4, E], F32, tag="res")
                    nc.sync.dma_start(out=res, in_=emb_res[ch])
                    fo = finp.tile([CH // 4, E], F32, tag="fo")
                    nc.vector.tensor_add(fo, res, fin)
                    nc.sync.dma_start(out=out_slice[ch], in_=fo)

    nc.finalize()
    return nc


def _prep_inputs(embeddings, cos_buffer, sin_buffer, causal_buffer,
                 w_norm, w_qkv, w_out, causal_tril):
    ks = K * H
    cq = np.ascontiguousarray(np.asarray(cos_buffer)[0, 0, 0].T)
    sq = np.ascontiguousarray(np.asarray(sin_buffer)[0, 0, 0].T)
    ck = np.ascontiguousarray(np.asarray(cos_buffer)[1, 0, 0].T)
    sk = np.ascontiguousarray(np.asarray(sin_buffer)[1, 0, 0].T)
    wn_t = np.ascontiguousarray(np.asarray(w_norm).reshape(ET, P).T)
    if not causal_tril:
        maskT = np.ascontiguousarray(
            np.asarray(causal_buffer)[0, 0].T.astype(np.float32))

    in_maps = []
    for c in range(8):
        b, g = c // 4, c % 4
        emb = np.asarray(embeddings)[b]
        embT = np.ascontiguousarray(emb.T)
        rq = CH // 4
        emb_res = np.stack([emb[CH * c + rq * g: CH * c + rq * (g + 1), :]
                            for c in range(NCH)])
        wq = np.asarray(w_qkv)
        w_qkv_t = np.ascontiguousarray(np.concatenate([
            wq[:, QC * g: QC * (g + 1)],
            wq[:, ks + QC * g: ks + QC * (g + 1)],
            wq[:, 2 * ks + QC * g: 2 * ks + QC * (g + 1)],
        ], axis=1))
        w_out_sl = np.ascontiguousarray(np.asarray(w_out)[QC * g: QC * (g + 1), :])
        m = dict(embT=embT, emb_res=np.ascontiguousarray(emb_res),
                 w_qkv_t=w_qkv_t, w_out_s=w_out_sl, w_norm_t=wn_t,
                 cos_q=cq, sin_q=sq, cos_k=ck, sin_k=sk)
        if not causal_tril:
            m["maskT"] = maskT
        in_maps.append(m)
    return in_maps


def kernel(embeddings, cos_buffer, sin_buffer, causal_buffer,
           w_norm, w_qkv, w_out, trace=False):
    global LAST_EXEC_NS, LAST_RESULTS
    causal = np.asarray(causal_buffer)[0, 0]
    causal_tril = bool(np.array_equal(causal, np.tril(np.ones((S, S), bool))))

    if causal_tril not in _CACHE:
        _CACHE[causal_tril] = build_graph(causal_tril)
    nc = _CACHE[causal_tril]

    in_maps = _prep_inputs(embeddings, cos_buffer, sin_buffer, causal_buffer,
                           w_norm, w_qkv, w_out, causal_tril)
    if trace:
        _ensure_ntff_hook()
    res = run_bass_kernel_spmd(nc, in_maps, core_ids=list(range(8)), trace=trace)
    LAST_EXEC_NS = res.exec_time_ns
    LAST_RESULTS = res

    out = np.empty((B, S, E), np.float32)
    present_k = np.empty((B, H, S, K), np.float32)
    present_v = np.empty((B, H, S, V), np.float32)
    for c in range(8):
        b, g = c // 4, c % 4
        r = res.results[c]
        osl = r["out_slice"]
        rq = CH // 4
        for c in range(NCH):
            out[b, CH * c + rq * g: CH * c + rq * (g + 1), :] = osl[c]
        ko = r["k_out"]          # [HL, K, S]
        vo = r["v_out"]          # [S, QC]
        for j in range(HL):
            present_k[b, HL * g + j] = ko[j].T
            present_v[b, HL * g + j] = vo[:, K * j: K * (j + 1)]
    return out, present_k, present_v


# revision 18
# speedup vs baseline: 1.7843x; 1.0883x over previous
"""Trainium2 Bass kernel for nn_AthenaSA: RMSNorm -> fused QKV -> RoPE ->
causal SDPA -> out_proj + residual, returning (out, present_k, present_v).

Sharding (8 cores): batch (2-way data parallel) x heads (4-way tensor
parallel).  Core c handles batch b=c//4 and heads [4g, 4g+4), g=c%4.  Each
core computes its 4 heads end-to-end; the out_proj partial sums are
reduce-scattered over each batch group of 4 cores, the residual is added to
the owned S/4 slice, and the host reassembles full outputs.

Compute is bf16 on the TensorEngine (fp32 PSUM accumulate); softmax sums and
normalization are fp32.  Softmax skips max-subtraction (scores are O(10) for
this problem's scale, safe in fp32 exp).

Attention layout trick: scores are computed TRANSPOSED (s_k on partitions,
s_q on free axis) so no transposes are needed anywhere: QK^T uses k_rot^T
tiles as the stationary operand, softmax denominators come from an extra
all-ones matmul accumulated alongside PV, and PV consumes v in natural
layout as the stationary operand, producing attn_out^T which feeds out_proj
directly.
"""
import math
import sys

import numpy as np

sys.path.insert(0, "/opt/trn_rl_repo")

import concourse.bass as bass  # noqa: E402
import concourse.tile as tile  # noqa: E402
from concourse import bacc, mybir  # noqa: E402
from concourse.bass_utils import run_bass_kernel_spmd  # noqa: E402

B, S, E, H, K, V = 2, 2048, 2048, 16, 128, 128
EPS = 1e-5
HL = 4            # heads per core
P = 128           # partitions
ET = E // P       # 16 e-tiles
ST = S // P       # 16 s-tiles
SB = 512          # s-block (psum free width)
NSB = S // SB     # 4 s-blocks
QC = HL * K       # 512 local qkv cols for each of q/k/v
CH = 256          # RS chunk rows
NCH = S // CH     # 8 RS chunks
F32 = mybir.dt.float32
BF16 = mybir.dt.bfloat16

_CACHE = {}
LAST_EXEC_NS = None
LAST_RESULTS = None


def _ensure_ntff_hook():
    """The image's antenv lacks axon_hooks; install an equivalent shim so
    run_bass_kernel_spmd(trace=True) can capture NTFF profiles."""
    import types
    try:
        from antenv.axon_hooks import get_axon_ntff_profile_hook  # noqa: F401
        return
    except ImportError:
        pass
    try:
        import antenv
        from trn_agent_boot.trn_boot import _ntff_profile_via_ctypes
        m = types.ModuleType("antenv.axon_hooks")
        m._hook = _ntff_profile_via_ctypes("/opt/axon/libaxon_pjrt.so")
        m.set_axon_ntff_profile_hook = lambda h: setattr(m, "_hook", h)
        m.get_axon_ntff_profile_hook = lambda: m._hook
        sys.modules["antenv.axon_hooks"] = m
        antenv.axon_hooks = m
    except Exception:
        pass


def build_graph(causal_tril: bool):
    nc = bacc.Bacc("TRN2", target_bir_lowering=False, debug=False, num_devices=8)

    embT = nc.dram_tensor("embT", [E, S], F32, kind="ExternalInput").ap()
    emb_res = nc.dram_tensor("emb_res", [NCH, CH // 4, E], F32, kind="ExternalInput").ap()
    w_qkv_t = nc.dram_tensor("w_qkv_t", [E, 3 * QC], F32, kind="ExternalInput").ap()
    w_out_s = nc.dram_tensor("w_out_s", [QC, E], F32, kind="ExternalInput").ap()
    w_norm_t = nc.dram_tensor("w_norm_t", [P, ET], F32, kind="ExternalInput").ap()
    cos_q = nc.dram_tensor("cos_q", [K, S], F32, kind="ExternalInput").ap()
    sin_q = nc.dram_tensor("sin_q", [K, S], F32, kind="ExternalInput").ap()
    cos_k = nc.dram_tensor("cos_k", [K, S], F32, kind="ExternalInput").ap()
    sin_k = nc.dram_tensor("sin_k", [K, S], F32, kind="ExternalInput").ap()
    if not causal_tril:
        maskT = nc.dram_tensor("maskT", [S, S], F32, kind="ExternalInput").ap()

    out_slice = nc.dram_tensor("out_slice", [NCH, CH // 4, E], F32,
                               kind="ExternalOutput").ap()
    k_out = nc.dram_tensor("k_out", [HL, K, S], F32, kind="ExternalOutput").ap()
    v_out = nc.dram_tensor("v_out", [S, QC], F32, kind="ExternalOutput").ap()

    inv_sqrt_k = 1.0 / math.sqrt(K)
    half = K // 2

    with tile.TileContext(nc) as tc:
        with (
            tc.tile_pool(name="dram", bufs=1, space="DRAM") as dram,
            tc.tile_pool(name="persist", bufs=1) as persist,
            tc.tile_pool(name="qkvout", bufs=1) as qkvout,
            tc.tile_pool(name="scalep", bufs=1) as scalep,
        ):
            partial = [dram.tile([CH, E], BF16, tag=f"partial{c}",
                                 name=f"partial{c}") for c in range(NCH)]
            rs_out_d = [dram.tile([CH // 4, E], BF16, tag=f"rs_out{c}",
                                  name=f"rs_out{c}") for c in range(NCH)]
            scale_d = dram.tile([1, S], F32, tag="scale_d")

            ones_bf = persist.tile([P, P], BF16, tag="ones")
            nc.vector.memset(ones_bf, 1.0)
            ones1 = persist.tile([P, 1], BF16, tag="ones1")
            nc.vector.memset(ones1, 1.0)
            eps_t = persist.tile([P, 1], F32, tag="eps")
            nc.vector.memset(eps_t, EPS)
            wn_sb = persist.tile([P, ET], F32, tag="wn")
            nc.sync.dma_start(out=wn_sb, in_=w_norm_t[:, :])
            scale_col = persist.tile([P, ST], F32, tag="scale_col")

            # RMS scale, broadcast across partitions: scale_bc[p, s] = rsqrt(ms[s])
            scale_bc = scalep.tile([P, S], F32, tag="scale")

            # q/k arrive transposed [head_dim, S]; RoPE is applied IN PLACE.
            q_bf = [qkvout.tile([P, S], BF16, tag=f"qbf{j}", name=f"qbf{j}")
                    for j in range(HL)]
            k_bf = [qkvout.tile([P, S], BF16, tag=f"kbf{j}", name=f"kbf{j}")
                    for j in range(HL)]
            v_bf = [qkvout.tile([P, QC], BF16, tag=f"vbf{m}", name=f"vbf{m}")
                    for m in range(ST)]

            # ====== Phases A+B: stats + QKV (scale factored out of matmuls),
            # ====== with RoPE interleaved against the v-projection.
            with tc.tile_pool(name="xpool", bufs=1) as xpool, \
                 tc.tile_pool(name="trig", bufs=1) as trig:
                cq = trig.tile([K, S], BF16, tag="cq")
                sq_ = trig.tile([K, S], BF16, tag="sq_")
                ck = trig.tile([K, S], BF16, tag="ck")
                sk = trig.tile([K, S], BF16, tag="sk")

                x = []
                with tc.tile_pool(name="xstage", bufs=3) as xstage, \
                     tc.tile_pool(name="sqpool", bufs=3) as sqpool, \
                     tc.tile_pool(name="mspool", bufs=1, space="PSUM") as mspool:
                    ms_ps = [mspool.tile([P, SB], F32, tag=f"ms{q}", name=f"ms{q}")
                             for q in range(NSB)]
                    for e in range(ET):
                        xs = xstage.tile([P, S], F32, tag="xs")
                        nc.sync.dma_start(out=xs, in_=embT[e * P:(e + 1) * P, :])
                        xt = xpool.tile([P, S], BF16, tag=f"x{e}", name=f"x{e}")
                        nc.vector.tensor_copy(out=xt, in_=xs)
                        x.append(xt)
                        sq = sqpool.tile([P, S], BF16, tag="sq")
                        nc.scalar.activation(out=sq, in_=xs,
                                             func=mybir.ActivationFunctionType.Square)
                        for q in range(NSB):
                            nc.tensor.matmul(ms_ps[q][:, :], ones_bf,
                                             sq[:, q * SB:(q + 1) * SB],
                                             start=(e == 0), stop=(e == ET - 1))

                    for dst, srcdram in ((cq, cos_q), (sq_, sin_q),
                                         (ck, cos_k), (sk, sin_k)):
                        ts_ = xstage.tile([K, S], F32, tag="xs", name="ts_")
                        nc.sync.dma_start(out=ts_, in_=srcdram[:, :])
                        nc.vector.tensor_copy(out=dst, in_=ts_)

                    for q in range(NSB):
                        rms = sqpool.tile([P, SB], F32, tag="rms", name="rms")
                        nc.scalar.activation(out=rms, in_=ms_ps[q][:, :],
                                             func=mybir.ActivationFunctionType.Sqrt,
                                             bias=eps_t, scale=1.0 / E)
                        nc.vector.reciprocal_approx_fast(
                            out=scale_bc[:, q * SB:(q + 1) * SB], in_=rms)
                    # scale in column layout (scale_col[p, m] = scale[m*128+p])
                    # via a small DRAM round trip
                    nc.sync.dma_start(out=scale_d[:, :], in_=scale_bc[0:1, :])
                    nc.sync.dma_start(
                        out=scale_col,
                        in_=scale_d[0].rearrange("(t p) -> p t", p=P))

                # ---- QKV projections; w_qkv streamed in 512-col slices with
                # ---- w_norm folded into the bf16 cast
                with tc.tile_pool(name="qkps", bufs=3, space="PSUM") as qkps, \
                     tc.tile_pool(name="qkcp", bufs=3) as qkcp, \
                     tc.tile_pool(name="ropetmp", bufs=2) as ropetmp:
                    def rope_inplace(src, c_, s_):
                        t1 = ropetmp.tile([K, S], BF16, tag="t1", name="t1")
                        nc.vector.tensor_copy(out=t1[0:half, :], in_=src[half:K, :])
                        nc.vector.tensor_copy(out=t1[half:K, :], in_=src[0:half, :])
                        nc.vector.tensor_mul(t1, t1, s_)
                        nc.vector.tensor_mul(src, src, c_)
                        nc.vector.tensor_add(src, src, t1)

                    for sl in range(3):      # 0: q cols, 1: k cols, 2: v cols
                        with tc.tile_pool(name=f"wp{sl}", bufs=1) as wpool:
                            w_bf = []
                            for e in range(ET):
                                ws = qkcp.tile([P, QC], F32, tag="wstage",
                                               name="ws")
                                nc.sync.dma_start(
                                    out=ws,
                                    in_=w_qkv_t[e * P:(e + 1) * P,
                                                sl * QC:(sl + 1) * QC])
                                wt = wpool.tile([P, QC], BF16, tag=f"w{e}",
                                                name=f"w{e}")
                                nc.vector.tensor_scalar_mul(wt, ws,
                                                            wn_sb[:, e:e + 1])
                                w_bf.append(wt)
                            if sl < 2:
                                dsts = q_bf if sl == 0 else k_bf
                                for j in range(HL):
                                    for q in range(NSB):
                                        sslc = slice(q * SB, (q + 1) * SB)
                                        ps = qkps.tile([P, SB], F32, tag="qk")
                                        for e in range(ET):
                                            nc.tensor.matmul(
                                                ps[:, :],
                                                w_bf[e][:, j * P:(j + 1) * P],
                                                x[e][:, sslc],
                                                start=(e == 0), stop=(e == ET - 1))
                                        nc.vector.tensor_mul(
                                            dsts[j][:, sslc], ps[:, :],
                                            scale_bc[:, sslc])
                                        if sl == 1:
                                            kf = qkcp.tile([P, SB], F32,
                                                           tag="kf", name="kf")
                                            nc.vector.tensor_mul(
                                                kf, ps[:, :], scale_bc[:, sslc])
                                            nc.sync.dma_start(
                                                out=k_out[j][:, sslc], in_=kf)
                                    if sl == 0:
                                        rope_inplace(q_bf[j], cq, sq_)
                                    else:
                                        rope_inplace(k_bf[j], ck, sk)
                            else:
                                for m in range(ST):
                                    ps = qkps.tile([P, QC], F32, tag="qk")
                                    for e in range(ET):
                                        nc.tensor.matmul(
                                            ps[:, :],
                                            x[e][:, m * P:(m + 1) * P],
                                            w_bf[e][:, :],
                                            start=(e == 0), stop=(e == ET - 1))
                                    vf = qkcp.tile([P, QC], F32, tag="vf")
                                    nc.vector.tensor_scalar_mul(
                                        vf, ps[:, :], scale_col[:, m:m + 1])
                                    nc.sync.dma_start(
                                        out=v_out[m * P:(m + 1) * P, :], in_=vf)
                                    nc.vector.tensor_copy(out=v_bf[m], in_=vf)

            # ====== Phase D: attention + out_proj + chunked RS + residual ===
            with tc.tile_pool(name="wout", bufs=1) as woutp, \
                 tc.tile_pool(name="maskp", bufs=1) as maskp, \
                 tc.tile_pool(name="attn", bufs=1) as attnp, \
                 tc.tile_pool(name="expp", bufs=4) as expp, \
                 tc.tile_pool(name="smallp", bufs=4) as smallp, \
                 tc.tile_pool(name="qk2ps", bufs=2, space="PSUM") as qk2ps, \
                 tc.tile_pool(name="pvps", bufs=2, space="PSUM") as pvps, \
                 tc.tile_pool(name="sumps", bufs=2, space="PSUM") as sumps, \
                 tc.tile_pool(name="opps", bufs=2, space="PSUM") as opps, \
                 tc.tile_pool(name="finp", bufs=2) as finp:

                w_out_bf = []
                for j in range(HL):
                    wos = finp.tile([P, E], F32, tag="wostage", name="wos")
                    nc.sync.dma_start(out=wos, in_=w_out_s[j * P:(j + 1) * P, :])
                    wt = woutp.tile([P, E], BF16, tag=f"wo{j}", name=f"wo{j}")
                    nc.vector.tensor_copy(out=wt, in_=wos)
                    w_out_bf.append(wt)

                if causal_tril:
                    masks = []
                    ones_m = maskp.tile([P, SB], BF16, tag="ones_m")
                    nc.vector.memset(ones_m, 1.0)
                    for d in range(NSB):
                        mk = maskp.tile([P, SB], BF16, tag=f"mk{d}",
                                        name=f"mk{d}")
                        nc.gpsimd.affine_select(
                            out=mk, in_=ones_m, pattern=[[1, SB]],
                            compare_op=mybir.AluOpType.is_ge, fill=0.0,
                            base=-P * d, channel_multiplier=-1)
                        masks.append(mk)
                else:
                    mask_bf = []
                    for t in range(ST):
                        mts = finp.tile([P, S], F32, tag="maskstage", name="mts")
                        nc.sync.dma_start(out=mts, in_=maskT[t * P:(t + 1) * P, :])
                        mt = maskp.tile([P, S], BF16, tag=f"mask{t}",
                                        name=f"mt{t}")
                        nc.vector.tensor_copy(out=mt, in_=mts)
                        mask_bf.append(mt)

                attn_bf = [[attnp.tile([P, SB], BF16, tag=f"attn{j}_{q}",
                                       name=f"attn{j}_{q}")
                            for q in range(NSB)] for j in range(HL)]

                for Q in range(NSB):
                    nt = 4 * Q + 4 if causal_tril else ST
                    for j in range(HL):
                        pv = pvps.tile([P, SB], F32, tag="pv")
                        sm = sumps.tile([P, SB], F32, tag="sm")
                        for t in range(nt):
                            qk = qk2ps.tile([P, SB], F32, tag="qk2")
                            nc.tensor.matmul(
                                qk[:, :],
                                k_bf[j][:, t * P:(t + 1) * P],
                                q_bf[j][:, Q * SB:(Q + 1) * SB],
                                start=True, stop=True)
                            ex = expp.tile([P, SB], BF16, tag="ex")
                            nc.scalar.activation(
                                out=ex, in_=qk[:, :],
                                func=mybir.ActivationFunctionType.Exp,
                                scale=inv_sqrt_k)
                            if causal_tril:
                                if t >= 4 * Q:
                                    nc.vector.tensor_mul(ex, ex,
                                                         masks[t - 4 * Q])
                            else:
                                nc.vector.tensor_mul(
                                    ex, ex, mask_bf[t][:, Q * SB:(Q + 1) * SB])
                            nc.tensor.matmul(pv[:, :],
                                             v_bf[t][:, j * K:(j + 1) * K], ex,
                                             start=(t == 0), stop=(t == nt - 1))
                            nc.tensor.matmul(sm[:, :], ones_bf, ex,
                                             start=(t == 0), stop=(t == nt - 1))
                        rc = smallp.tile([P, SB], F32, tag="rc")
                        nc.vector.reciprocal_approx_fast(out=rc, in_=sm[:, :])
                        nc.vector.tensor_mul(attn_bf[j][Q], pv[:, :], rc)

                    # out_proj rows [512Q, 512Q+512); RS fires every 256 rows
                    for m in range(NSB):
                        for eb in range(NSB):
                            op = opps.tile([P, SB], F32, tag="op")
                            for j in range(HL):
                                nc.tensor.matmul(
                                    op[:, :],
                                    attn_bf[j][Q][:, m * P:(m + 1) * P],
                                    w_out_bf[j][:, eb * SB:(eb + 1) * SB],
                                    start=(j == 0), stop=(j == HL - 1))
                            ob = smallp.tile([P, SB], BF16, tag="ob")
                            nc.vector.tensor_copy(out=ob, in_=op[:, :])
                            ch = (Q * SB + m * P) // CH
                            row = (Q * SB + m * P) % CH
                            nc.sync.dma_start(
                                out=partial[ch][row:row + P,
                                               eb * SB:(eb + 1) * SB],
                                in_=ob)
                        if (m * P + P) % CH == 0:
                            ch = (Q * SB + m * P) // CH
                            nc.gpsimd.collective_compute(
                                "ReduceScatter",
                                mybir.AluOpType.add,
                                ins=[partial[ch][:, :]],
                                outs=[rs_out_d[ch][:, :]],
                                replica_groups=[[0, 1, 2, 3], [4, 5, 6, 7]],
                            )

                # residual + writeback after all compute: keeps the
                # RS-latency waits off the in-order engine streams
                for ch in range(NCH):
                    fin = finp.tile([CH // 4, E], BF16, tag="fin")
                    nc.sync.dma_start(out=fin, in_=rs_out_d[ch][:, :])
                    res = finp.tile([CH // 4, E], F32, tag="res")
                    nc.sync.dma_start(out=res, in_=emb_res[ch])
                    fo = finp.tile([CH // 4, E], F32, tag="fo")
                    nc.vector.tensor_add(fo, res, fin)
                    nc.sync.dma_start(out=out_slice[ch], in_=fo)

    nc.finalize()
    return nc


def _prep_inputs(embeddings, cos_buffer, sin_buffer, causal_buffer,
                 w_norm, w_qkv, w_out, causal_tril):
    ks = K * H
    cq = np.ascontiguousarray(np.asarray(cos_buffer)[0, 0, 0].T)
    sq = np.ascontiguousarray(np.asarray(sin_buffer)[0, 0, 0].T)
    ck = np.ascontiguousarray(np.asarray(cos_buffer)[1, 0, 0].T)
    sk = np.ascontiguousarray(np.asarray(sin_buffer)[1, 0, 0].T)
    wn_t = np.ascontiguousarray(np.asarray(w_norm).reshape(ET, P).T)
    if not causal_tril:
        maskT = np.ascontiguousarray(
            np.asarray(causal_buffer)[0, 0].T.astype(np.float32))

    in_maps = []
    for c in range(8):
        b, g = c // 4, c % 4
        emb = np.asarray(embeddings)[b]
        embT = np.ascontiguousarray(emb.T)
        rq = CH // 4
        emb_res = np.stack([emb[CH * c + rq * g: CH * c + rq * (g + 1), :]
                            for c in range(NCH)])
        wq = np.asarray(w_qkv)
        w_qkv_t = np.ascontiguousarray(np.concatenate([
            wq[:, QC * g: QC * (g + 1)],
            wq[:, ks + QC * g: ks + QC * (g + 1)],
            wq[:, 2 * ks + QC * g: 2 * ks + QC * (g + 1)],
        ], axis=1))
        w_out_sl = np.ascontiguousarray(np.asarray(w_out)[QC * g: QC * (g + 1), :])
        m = dict(embT=embT, emb_res=np.ascontiguousarray(emb_res),
                 w_qkv_t=w_qkv_t, w_out_s=w_out_sl, w_norm_t=wn_t,
                 cos_q=cq, sin_q=sq, cos_k=ck, sin_k=sk)
        if not causal_tril:
            m["maskT"] = maskT
        in_maps.append(m)
    return in_maps


def kernel(embeddings, cos_buffer, sin_buffer, causal_buffer,
           w_norm, w_qkv, w_out, trace=False):
    global LAST_EXEC_NS, LAST_RESULTS
    causal = np.asarray(causal_buffer)[0, 0]
    causal_tril = bool(np.array_equal(causal, np.tril(np.ones((S, S), bool))))

    if causal_tril not in _CACHE:
        _CACHE[causal_tril] = build_graph(causal_tril)
    nc = _CACHE[causal_tril]

    in_maps = _prep_inputs(embeddings, cos_buffer, sin_buffer, causal_buffer,
                           w_norm, w_qkv, w_out, causal_tril)
    if trace:
        _ensure_ntff_hook()
    res = run_bass_kernel_spmd(nc, in_maps, core_ids=list(range(8)), trace=trace)
    LAST_EXEC_NS = res.exec_time_ns
    LAST_RESULTS = res

    out = np.empty((B, S, E), np.float32)
    present_k = np.empty((B, H, S, K), np.float32)
    present_v = np.empty((B, H, S, V), np.float32)
    for c in range(8):
        b, g = c // 4, c % 4
        r = res.results[c]
        osl = r["out_slice"]
        rq = CH // 4
        for c in range(NCH):
            out[b, CH * c + rq * g: CH * c + rq * (g + 1), :] = osl[c]
        ko = r["k_out"]          # [HL, K, S]
        vo = r["v_out"]          # [S, QC]
        for j in range(HL):
            present_k[b, HL * g + j] = ko[j].T
            present_v[b, HL * g + j] = vo[:, K * j: K * (j + 1)]
    return out, present_k, present_v


# revision 20
# speedup vs baseline: 1.9369x; 1.0855x over previous
"""Trainium2 Bass kernel for nn_AthenaSA: RMSNorm -> fused QKV -> RoPE ->
causal SDPA -> out_proj + residual, returning (out, present_k, present_v).

Sharding (8 cores): batch (2-way data parallel) x heads (4-way tensor
parallel).  Core c handles batch b=c//4 and heads [4g, 4g+4), g=c%4.  Each
core computes its 4 heads end-to-end; the out_proj partial sums are
reduce-scattered over each batch group of 4 cores, the residual is added to
the owned S/4 slice, and the host reassembles full outputs.

Compute is bf16 on the TensorEngine (fp32 PSUM accumulate); softmax sums and
normalization are fp32.  Softmax skips max-subtraction (scores are O(10) for
this problem's scale, safe in fp32 exp).

Attention layout trick: scores are computed TRANSPOSED (s_k on partitions,
s_q on free axis) so no transposes are needed anywhere: QK^T uses k_rot^T
tiles as the stationary operand, softmax denominators come from an extra
all-ones matmul accumulated alongside PV, and PV consumes v in natural
layout as the stationary operand, producing attn_out^T which feeds out_proj
directly.
"""
import math
import sys

import numpy as np

sys.path.insert(0, "/opt/trn_rl_repo")

import concourse.bass as bass  # noqa: E402
import concourse.tile as tile  # noqa: E402
from concourse import bacc, mybir  # noqa: E402
from concourse.bass_utils import run_bass_kernel_spmd  # noqa: E402

B, S, E, H, K, V = 2, 2048, 2048, 16, 128, 128
EPS = 1e-5
HL = 4            # heads per core
P = 128           # partitions
ET = E // P       # 16 e-tiles
ST = S // P       # 16 s-tiles
SB = 512          # s-block (psum free width)
NSB = S // SB     # 4 s-blocks
QC = HL * K       # 512 local qkv cols for each of q/k/v
CH = 256          # RS chunk rows
NCH = S // CH     # 8 RS chunks
F32 = mybir.dt.float32
BF16 = mybir.dt.bfloat16

_CACHE = {}
LAST_EXEC_NS = None
LAST_RESULTS = None


def _ensure_ntff_hook():
    """The image's antenv lacks axon_hooks; install an equivalent shim so
    run_bass_kernel_spmd(trace=True) can capture NTFF profiles."""
    import types
    try:
        from antenv.axon_hooks import get_axon_ntff_profile_hook  # noqa: F401
        return
    except ImportError:
        pass
    try:
        import antenv
        from trn_agent_boot.trn_boot import _ntff_profile_via_ctypes
        m = types.ModuleType("antenv.axon_hooks")
        m._hook = _ntff_profile_via_ctypes("/opt/axon/libaxon_pjrt.so")
        m.set_axon_ntff_profile_hook = lambda h: setattr(m, "_hook", h)
        m.get_axon_ntff_profile_hook = lambda: m._hook
        sys.modules["antenv.axon_hooks"] = m
        antenv.axon_hooks = m
    except Exception:
        pass


# RS chunking: (row0, nrows) per collective; finer at the tail so the last
# chunk's latency exposure is small
CHUNKS = [(0, 256), (256, 256), (512, 256), (768, 256), (1024, 256),
          (1280, 256), (1536, 256), (1792, 128), (1920, 128)]


def build_graph(causal_tril: bool):
    nc = bacc.Bacc("TRN2", target_bir_lowering=False, debug=False, num_devices=8)

    OWN = S // 4
    embT = nc.dram_tensor("embT", [E, S], F32, kind="ExternalInput").ap()
    emb_res = nc.dram_tensor("emb_res", [OWN, E], F32, kind="ExternalInput").ap()
    w_qkv_t = nc.dram_tensor("w_qkv_t", [E, 3 * QC], F32, kind="ExternalInput").ap()
    w_out_s = nc.dram_tensor("w_out_s", [QC, E], F32, kind="ExternalInput").ap()
    w_norm_t = nc.dram_tensor("w_norm_t", [P, ET], F32, kind="ExternalInput").ap()
    cos_q = nc.dram_tensor("cos_q", [K, S], F32, kind="ExternalInput").ap()
    sin_q = nc.dram_tensor("sin_q", [K, S], F32, kind="ExternalInput").ap()
    cos_k = nc.dram_tensor("cos_k", [K, S], F32, kind="ExternalInput").ap()
    sin_k = nc.dram_tensor("sin_k", [K, S], F32, kind="ExternalInput").ap()
    if not causal_tril:
        maskT = nc.dram_tensor("maskT", [S, S], F32, kind="ExternalInput").ap()

    out_slice = nc.dram_tensor("out_slice", [OWN, E], F32,
                               kind="ExternalOutput").ap()
    k_out = nc.dram_tensor("k_out", [HL, K, S], F32, kind="ExternalOutput").ap()
    v_out = nc.dram_tensor("v_out", [S, QC], F32, kind="ExternalOutput").ap()

    inv_sqrt_k = 1.0 / math.sqrt(K)
    half = K // 2
    NC_ = len(CHUNKS)

    with tile.TileContext(nc) as tc:
        with (
            tc.tile_pool(name="dram", bufs=1, space="DRAM") as dram,
            tc.tile_pool(name="persist", bufs=1) as persist,
            tc.tile_pool(name="qkvout", bufs=1) as qkvout,
            tc.tile_pool(name="scalep", bufs=1) as scalep,
        ):
            partial = [dram.tile([nr, E], BF16, tag=f"partial{c}",
                                 name=f"partial{c}")
                       for c, (r0, nr) in enumerate(CHUNKS)]
            rs_out_d = [dram.tile([nr // 4, E], BF16, tag=f"rs_out{c}",
                                  name=f"rs_out{c}")
                        for c, (r0, nr) in enumerate(CHUNKS)]
            scale_d = dram.tile([1, S], F32, tag="scale_d")

            ones_bf = persist.tile([P, P], BF16, tag="ones")
            nc.vector.memset(ones_bf, 1.0)
            eps_t = persist.tile([P, 1], F32, tag="eps")
            nc.vector.memset(eps_t, EPS)
            wn_sb = persist.tile([P, ET], F32, tag="wn")
            nc.sync.dma_start(out=wn_sb, in_=w_norm_t[:, :])
            scale_col = persist.tile([P, ST], F32, tag="scale_col")

            scale_bc = scalep.tile([P, S], F32, tag="scale")

            q_bf = [qkvout.tile([P, S], BF16, tag=f"qbf{j}", name=f"qbf{j}")
                    for j in range(HL)]
            k_bf = [qkvout.tile([P, S], BF16, tag=f"kbf{j}", name=f"kbf{j}")
                    for j in range(HL)]
            v_bf = [qkvout.tile([P, QC], BF16, tag=f"vbf{m}", name=f"vbf{m}")
                    for m in range(ST)]

            # ====== Phases A+B: stats + QKV, RoPE interleaved =============
            with tc.tile_pool(name="xpool", bufs=1) as xpool, \
                 tc.tile_pool(name="wp0", bufs=1) as wp0, \
                 tc.tile_pool(name="trig", bufs=1) as trig:
                cq = trig.tile([K, S], BF16, tag="cq")
                sq_ = trig.tile([K, S], BF16, tag="sq_")
                ck = trig.tile([K, S], BF16, tag="ck")
                sk = trig.tile([K, S], BF16, tag="sk")

                x = []
                w_bf0 = []
                with tc.tile_pool(name="xstage", bufs=3) as xstage, \
                     tc.tile_pool(name="sqpool", bufs=3) as sqpool, \
                     tc.tile_pool(name="mspool", bufs=1, space="PSUM") as mspool:
                    ms_ps = [mspool.tile([P, SB], F32, tag=f"ms{q}", name=f"ms{q}")
                             for q in range(NSB)]
                    # interleave x loads with w-slice-0 loads so the q-proj can
                    # start the moment the last x tile lands
                    for e in range(ET):
                        xs = xstage.tile([P, S], F32, tag="xs")
                        nc.sync.dma_start(out=xs, in_=embT[e * P:(e + 1) * P, :])
                        xt = xpool.tile([P, S], BF16, tag=f"x{e}", name=f"x{e}")
                        nc.vector.tensor_copy(out=xt, in_=xs)
                        x.append(xt)
                        sq = sqpool.tile([P, S], BF16, tag="sq")
                        nc.scalar.activation(out=sq, in_=xs,
                                             func=mybir.ActivationFunctionType.Square)
                        for q in range(NSB):
                            nc.tensor.matmul(ms_ps[q][:, :], ones_bf,
                                             sq[:, q * SB:(q + 1) * SB],
                                             start=(e == 0), stop=(e == ET - 1))
                        ws = sqpool.tile([P, QC], F32, tag="ws0", name="ws0",
                                         bufs=2)
                        nc.sync.dma_start(out=ws, in_=w_qkv_t[e * P:(e + 1) * P,
                                                             0:QC])
                        wt = wp0.tile([P, QC], BF16, tag=f"w0_{e}",
                                      name=f"w0_{e}")
                        nc.vector.tensor_scalar_mul(wt, ws, wn_sb[:, e:e + 1])
                        w_bf0.append(wt)

                    for dst, srcdram in ((cq, cos_q), (sq_, sin_q),
                                         (ck, cos_k), (sk, sin_k)):
                        ts_ = xstage.tile([K, S], F32, tag="xs", name="ts_")
                        nc.sync.dma_start(out=ts_, in_=srcdram[:, :])
                        nc.vector.tensor_copy(out=dst, in_=ts_)

                    for q in range(NSB):
                        rms = sqpool.tile([P, SB], F32, tag="rms", name="rms")
                        nc.scalar.activation(out=rms, in_=ms_ps[q][:, :],
                                             func=mybir.ActivationFunctionType.Sqrt,
                                             bias=eps_t, scale=1.0 / E)
                        nc.vector.reciprocal_approx_fast(
                            out=scale_bc[:, q * SB:(q + 1) * SB], in_=rms)
                    nc.sync.dma_start(out=scale_d[:, :], in_=scale_bc[0:1, :])
                    nc.sync.dma_start(
                        out=scale_col,
                        in_=scale_d[0].rearrange("(t p) -> p t", p=P))

                with tc.tile_pool(name="qkps", bufs=3, space="PSUM") as qkps, \
                     tc.tile_pool(name="qkcp", bufs=3) as qkcp, \
                     tc.tile_pool(name="ropetmp", bufs=2) as ropetmp:
                    def rope_inplace(src, c_, s_):
                        t1 = ropetmp.tile([K, S], BF16, tag="t1", name="t1")
                        nc.vector.tensor_copy(out=t1[0:half, :], in_=src[half:K, :])
                        nc.vector.tensor_copy(out=t1[half:K, :], in_=src[0:half, :])
                        nc.vector.tensor_mul(t1, t1, s_)
                        nc.vector.tensor_mul(src, src, c_)
                        nc.vector.tensor_add(src, src, t1)

                    def qk_proj(w_bf, dsts, emit_k):
                        for j in range(HL):
                            for q in range(NSB):
                                sslc = slice(q * SB, (q + 1) * SB)
                                ps = qkps.tile([P, SB], F32, tag="qk")
                                for e in range(ET):
                                    nc.tensor.matmul(
                                        ps[:, :],
                                        w_bf[e][:, j * P:(j + 1) * P],
                                        x[e][:, sslc],
                                        start=(e == 0), stop=(e == ET - 1))
                                nc.vector.tensor_mul(
                                    dsts[j][:, sslc], ps[:, :],
                                    scale_bc[:, sslc])
                                if emit_k:
                                    kf = qkcp.tile([P, SB], F32,
                                                   tag="kf", name="kf")
                                    nc.vector.tensor_mul(
                                        kf, ps[:, :], scale_bc[:, sslc])
                                    nc.sync.dma_start(
                                        out=k_out[j][:, sslc], in_=kf)
                            if emit_k:
                                rope_inplace(k_bf[j], ck, sk)
                            else:
                                rope_inplace(q_bf[j], cq, sq_)

                    qk_proj(w_bf0, q_bf, False)

                    for sl in (1, 2):
                        with tc.tile_pool(name=f"wp{sl}", bufs=1) as wpool:
                            w_bf = []
                            for e in range(ET):
                                ws = qkcp.tile([P, QC], F32, tag="wstage",
                                               name="ws")
                                nc.sync.dma_start(
                                    out=ws,
                                    in_=w_qkv_t[e * P:(e + 1) * P,
                                                sl * QC:(sl + 1) * QC])
                                wt = wpool.tile([P, QC], BF16, tag=f"w{e}",
                                                name=f"w{e}")
                                nc.vector.tensor_scalar_mul(wt, ws,
                                                            wn_sb[:, e:e + 1])
                                w_bf.append(wt)
                            if sl == 1:
                                qk_proj(w_bf, k_bf, True)
                            else:
                                for m in range(ST):
                                    ps = qkps.tile([P, QC], F32, tag="qk")
                                    for e in range(ET):
                                        nc.tensor.matmul(
                                            ps[:, :],
                                            x[e][:, m * P:(m + 1) * P],
                                            w_bf[e][:, :],
                                            start=(e == 0), stop=(e == ET - 1))
                                    vf = qkcp.tile([P, QC], F32, tag="vf")
                                    nc.vector.tensor_scalar_mul(
                                        vf, ps[:, :], scale_col[:, m:m + 1])
                                    nc.sync.dma_start(
                                        out=v_out[m * P:(m + 1) * P, :], in_=vf)
                                    nc.vector.tensor_copy(out=v_bf[m], in_=vf)

            # ====== Phase D: attention + out_proj + chunked RS + residual ===
            with tc.tile_pool(name="wout", bufs=1) as woutp, \
                 tc.tile_pool(name="maskp", bufs=1) as maskp, \
                 tc.tile_pool(name="attn", bufs=1) as attnp, \
                 tc.tile_pool(name="expp", bufs=4) as expp, \
                 tc.tile_pool(name="smallp", bufs=4) as smallp, \
                 tc.tile_pool(name="qk2ps", bufs=2, space="PSUM") as qk2ps, \
                 tc.tile_pool(name="pvps", bufs=2, space="PSUM") as pvps, \
                 tc.tile_pool(name="sumps", bufs=2, space="PSUM") as sumps, \
                 tc.tile_pool(name="opps", bufs=2, space="PSUM") as opps, \
                 tc.tile_pool(name="finp", bufs=2) as finp:

                w_out_bf = []
                for j in range(HL):
                    wos = finp.tile([P, E], F32, tag="wostage", name="wos")
                    nc.sync.dma_start(out=wos, in_=w_out_s[j * P:(j + 1) * P, :])
                    wt = woutp.tile([P, E], BF16, tag=f"wo{j}", name=f"wo{j}")
                    nc.vector.tensor_copy(out=wt, in_=wos)
                    w_out_bf.append(wt)

                if causal_tril:
                    # single 128x128 lower-triangle mask for the diagonal square
                    ones_m = maskp.tile([P, P], BF16, tag="ones_m")
                    nc.vector.memset(ones_m, 1.0)
                    trimask = maskp.tile([P, P], BF16, tag="trimask")
                    nc.gpsimd.affine_select(
                        out=trimask, in_=ones_m, pattern=[[1, P]],
                        compare_op=mybir.AluOpType.is_ge, fill=0.0,
                        base=0, channel_multiplier=-1)
                else:
                    mask_bf = []
                    for t in range(ST):
                        mts = finp.tile([P, S], F32, tag="maskstage", name="mts")
                        nc.sync.dma_start(out=mts, in_=maskT[t * P:(t + 1) * P, :])
                        mt = maskp.tile([P, S], BF16, tag=f"mask{t}",
                                        name=f"mt{t}")
                        nc.vector.tensor_copy(out=mt, in_=mts)
                        mask_bf.append(mt)

                attn_bf = [[attnp.tile([P, SB], BF16, tag=f"attn{j}_{q}",
                                       name=f"attn{j}_{q}")
                            for q in range(NSB)] for j in range(HL)]

                rs_issued = 0
                res_done = 0

                def emit_residual(ch):
                    r0, nr = CHUNKS[ch]
                    nr4 = nr // 4
                    off = r0 // 4
                    fin = finp.tile([64, E], BF16, tag="fin")
                    nc.sync.dma_start(out=fin[:nr4, :], in_=rs_out_d[ch][:, :])
                    res = finp.tile([64, E], F32, tag="res")
                    nc.sync.dma_start(out=res[:nr4, :],
                                      in_=emb_res[off:off + nr4, :])
                    fo = finp.tile([64, E], F32, tag="fo")
                    nc.vector.tensor_add(fo[:nr4, :], res[:nr4, :],
                                         fin[:nr4, :])
                    nc.sync.dma_start(out=out_slice[off:off + nr4, :],
                                      in_=fo[:nr4, :])

                for Q in range(NSB):
                    # residual chains for chunks whose RS completed two
                    # blocks ago -- keeps RS waits off the engine streams
                    while Q >= 2 and res_done < rs_issued - 2:
                        emit_residual(res_done)
                        res_done += 1

                    nt = 4 * Q + 4 if causal_tril else ST
                    for j in range(HL):
                        pv = pvps.tile([P, SB], F32, tag="pv")
                        sm = sumps.tile([P, SB], F32, tag="sm")
                        for t in range(nt):
                            if causal_tril and t >= 4 * Q:
                                tl = t - 4 * Q          # 0..3 diagonal square
                                c0 = tl * P             # first valid col
                            else:
                                tl = None
                                c0 = 0
                            nw = SB - c0
                            qk = qk2ps.tile([P, SB], F32, tag="qk2")
                            nc.tensor.matmul(
                                qk[:, 0:nw],
                                k_bf[j][:, t * P:(t + 1) * P],
                                q_bf[j][:, Q * SB + c0:(Q + 1) * SB],
                                start=True, stop=True)
                            ex = expp.tile([P, SB], BF16, tag="ex")
                            nc.scalar.activation(
                                out=ex[:, 0:nw], in_=qk[:, 0:nw],
                                func=mybir.ActivationFunctionType.Exp,
                                scale=inv_sqrt_k)
                            if tl is not None:
                                nc.vector.tensor_mul(ex[:, 0:P], ex[:, 0:P],
                                                     trimask)
                            elif not causal_tril:
                                nc.vector.tensor_mul(
                                    ex[:, 0:nw], ex[:, 0:nw],
                                    mask_bf[t][:, Q * SB:(Q + 1) * SB])
                            nc.tensor.matmul(pv[:, c0:SB],
                                             v_bf[t][:, j * K:(j + 1) * K],
                                             ex[:, 0:nw],
                                             start=(t == 0), stop=(t == nt - 1))
                            nc.tensor.matmul(sm[:, c0:SB], ones_bf, ex[:, 0:nw],
                                             start=(t == 0), stop=(t == nt - 1))
                        rc = smallp.tile([P, SB], F32, tag="rc")
                        nc.vector.reciprocal_approx_fast(out=rc, in_=sm[:, :])
                        nc.vector.tensor_mul(attn_bf[j][Q], pv[:, :], rc)

                    # out_proj rows [512Q, 512Q+512); RS per chunk boundary
                    for m in range(NSB):
                        row_g = Q * SB + m * P
                        for eb in range(NSB):
                            op = opps.tile([P, SB], F32, tag="op")
                            for j in range(HL):
                                nc.tensor.matmul(
                                    op[:, :],
                                    attn_bf[j][Q][:, m * P:(m + 1) * P],
                                    w_out_bf[j][:, eb * SB:(eb + 1) * SB],
                                    start=(j == 0), stop=(j == HL - 1))
                            ob = smallp.tile([P, SB], BF16, tag="ob")
                            nc.vector.tensor_copy(out=ob, in_=op[:, :])
                            ch0_, nr0_ = CHUNKS[rs_issued]
                            nc.sync.dma_start(
                                out=partial[rs_issued][row_g - ch0_:
                                                       row_g - ch0_ + P,
                                                       eb * SB:(eb + 1) * SB],
                                in_=ob)
                        r0, nr = CHUNKS[rs_issued]
                        if row_g + P == r0 + nr:
                            nc.gpsimd.collective_compute(
                                "ReduceScatter",
                                mybir.AluOpType.add,
                                ins=[partial[rs_issued][:, :]],
                                outs=[rs_out_d[rs_issued][:, :]],
                                replica_groups=[[0, 1, 2, 3], [4, 5, 6, 7]],
                            )
                            rs_issued += 1

                while res_done < NC_:
                    emit_residual(res_done)
                    res_done += 1

    nc.finalize()
    return nc


def _prep_inputs(embeddings, cos_buffer, sin_buffer, causal_buffer,
                 w_norm, w_qkv, w_out, causal_tril):
    ks = K * H
    cq = np.ascontiguousarray(np.asarray(cos_buffer)[0, 0, 0].T)
    sq = np.ascontiguousarray(np.asarray(sin_buffer)[0, 0, 0].T)
    ck = np.ascontiguousarray(np.asarray(cos_buffer)[1, 0, 0].T)
    sk = np.ascontiguousarray(np.asarray(sin_buffer)[1, 0, 0].T)
    wn_t = np.ascontiguousarray(np.asarray(w_norm).reshape(ET, P).T)
    if not causal_tril:
        maskT = np.ascontiguousarray(
            np.asarray(causal_buffer)[0, 0].T.astype(np.float32))

    in_maps = []
    for c in range(8):
        b, g = c // 4, c % 4
        emb = np.asarray(embeddings)[b]
        embT = np.ascontiguousarray(emb.T)
        emb_res = np.concatenate(
            [emb[r0 + (nr // 4) * g: r0 + (nr // 4) * (g + 1), :]
             for (r0, nr) in CHUNKS], axis=0)
        wq = np.asarray(w_qkv)
        w_qkv_t = np.ascontiguousarray(np.concatenate([
            wq[:, QC * g: QC * (g + 1)],
            wq[:, ks + QC * g: ks + QC * (g + 1)],
            wq[:, 2 * ks + QC * g: 2 * ks + QC * (g + 1)],
        ], axis=1))
        w_out_sl = np.ascontiguousarray(np.asarray(w_out)[QC * g: QC * (g + 1), :])
        m = dict(embT=embT, emb_res=np.ascontiguousarray(emb_res),
                 w_qkv_t=w_qkv_t, w_out_s=w_out_sl, w_norm_t=wn_t,
                 cos_q=cq, sin_q=sq, cos_k=ck, sin_k=sk)
        if not causal_tril:
            m["maskT"] = maskT
        in_maps.append(m)
    return in_maps


def kernel(embeddings, cos_buffer, sin_buffer, causal_buffer,
           w_norm, w_qkv, w_out, trace=False):
    global LAST_EXEC_NS, LAST_RESULTS
    causal = np.asarray(causal_buffer)[0, 0]
    causal_tril = bool(np.array_equal(causal, np.tril(np.ones((S, S), bool))))

    if causal_tril not in _CACHE:
        _CACHE[causal_tril] = build_graph(causal_tril)
    nc = _CACHE[causal_tril]

    in_maps = _prep_inputs(embeddings, cos_buffer, sin_buffer, causal_buffer,
                           w_norm, w_qkv, w_out, causal_tril)
    if trace:
        _ensure_ntff_hook()
    res = run_bass_kernel_spmd(nc, in_maps, core_ids=list(range(8)), trace=trace)
    LAST_EXEC_NS = res.exec_time_ns
    LAST_RESULTS = res

    out = np.empty((B, S, E), np.float32)
    present_k = np.empty((B, H, S, K), np.float32)
    present_v = np.empty((B, H, S, V), np.float32)
    for c in range(8):
        b, g = c // 4, c % 4
        r = res.results[c]
        osl = r["out_slice"]
        for (r0, nr) in CHUNKS:
            nr4 = nr // 4
            out[b, r0 + nr4 * g: r0 + nr4 * (g + 1), :] = osl[r0 // 4:
                                                              r0 // 4 + nr4]
        ko = r["k_out"]          # [HL, K, S]
        vo = r["v_out"]          # [S, QC]
        for j in range(HL):
            present_k[b, HL * g + j] = ko[j].T
            present_v[b, HL * g + j] = vo[:, K * j: K * (j + 1)]
    return out, present_k, present_v
